# revision 1
# baseline (speedup 1.0000x reference)
"""Trainium2 Bass kernel for C2f-with-DeformableAttention block.

Sharding: data-parallel over batch (8 images -> 8 NeuronCores), weights
replicated, no collectives. Each core runs the full block for one image:
  cv1 (1x1) -> split a/b -> 2x Bottleneck(3x3+3x3) -> msdeform attn
  -> concat(a,b,b1,b2,attn) -> cv2 (1x1), SiLU after every conv.

Per-core layouts:
  feature maps: channel-major [C partitions, H*W free]; 3x3-conv inputs are
  zero-padded [C, 66*66] so the 9 taps are contiguous shifted reads feeding
  PSUM-accumulated matmuls.
  deformable sampling: a per-head V4 table in DRAM packs the 4 bilinear
  corner pixels per base index into 512B rows, fetched by SWDGE dma_gather;
  bilinear + softmax attention weights fold into 16 coefficients per sample
  applied with one multiply + one strided reduce on VectorE. Border clipping
  is folded into the coefficients (base index clamped to [0,62]^2, weights
  remapped/zeroed), matching grid_sample(align_corners=False) + masking.
Matmuls run float32r (full-rate fp32 PE mode, fp32 PSUM accumulation); the
concat/cv2 and attention-output paths are bf16. SBUF pressure is managed
with two sequential pool scopes (convs, attention) plus tag-based slot
rotation for the large maps.
"""

import os
import sys

sys.path.insert(0, "/opt/trn_rl_repo")

import numpy as np

import concourse.bass as bass
import concourse.tile as tile
from concourse import bacc, mybir
from concourse.bass import AP
from concourse.bass_utils import run_bass_kernel_spmd
from concourse.masks import make_identity

F32 = mybir.dt.float32
F32R = mybir.dt.float32r
BF16 = mybir.dt.bfloat16
I16 = mybir.dt.int16
ALU = mybir.AluOpType
ACTF = mybir.ActivationFunctionType
AX = mybir.AxisListType

B, C1, C2 = 8, 512, 512
C = 256
D = 256
NH, NP = 8, 4
H = W = 64
L = H * W            # 4096
DH = D // NH         # 32
PW = W + 2           # 66
DOFF = 1             # leading pad element so tap offset -1 stays in-tile
PADLEN = PW * 66 + 16   # per-channel padded map length (+DOFF+tail slack)
OUTREG = 64 * PW     # 4224: contiguous output region = rows 1..64 (all cols)
V4ROWS = 4168        # 66 front pad + 4096 rows + tail
LT = L // 128        # 32
HLT = LT // 2        # 16 (coords run in two l-halves)
NT = L // 512        # 8

SIM_ACT = os.environ.get("BASS_KERNEL_SIM_ACT", "") == "sigmoid"
PH = int(os.environ.get("BASS_KERNEL_PHASES", "9"))
ACT_MAIN = ACTF.Sigmoid if SIM_ACT else ACTF.Silu

_cache = {}


def _ap(t, offset, dims):
    """AP into a DRAM tensor handle at element offset."""
    return AP(t.ap().tensor, offset, dims)


def _tap(tile_, offset, dims):
    """AP into an SBUF/DRAM tile at element offset from tile base."""
    a = tile_[:]
    return AP(a.tensor, a.offset + offset, dims)


def build(n_cores=8):
    key = ("nc", SIM_ACT, PH)
    if key in _cache:
        return _cache[key]
    nc = bacc.Bacc("TRN2", target_bir_lowering=False, debug=False,
                   num_devices=n_cores)

    xd = nc.dram_tensor("x", [C1, L], F32R, kind="ExternalInput")
    rbd = nc.dram_tensor("refer", [L, 2], F32, kind="ExternalInput")
    w1d = nc.dram_tensor("w1t", [C1, C1], F32R, kind="ExternalInput")
    wcd = nc.dram_tensor("wc", [4, 9, C, C], F32R, kind="ExternalInput")
    w2d = nc.dram_tensor("w2t", [5 * C, C2], BF16, kind="ExternalInput")
    vpd = nc.dram_tensor("vproj_w", [D, D], F32R, kind="ExternalInput")
    oad = nc.dram_tensor("offaw_w", [D, 96], F32R, kind="ExternalInput")
    owd = nc.dram_tensor("out_w", [D, D], BF16, kind="ExternalInput")
    vbd = nc.dram_tensor("vproj_b", [1, D], F32R, kind="ExternalInput")
    obd = nc.dram_tensor("offaw_b", [1, 96], F32R, kind="ExternalInput")
    wbd = nc.dram_tensor("out_b", [D, 1], F32, kind="ExternalInput")
    outd = nc.dram_tensor("out", [C2, L], F32, kind="ExternalOutput")

    with tile.TileContext(nc) as tc:
        _build_tile(nc, tc, xd, rbd, w1d, wcd, w2d, vpd, oad, owd, vbd, obd,
                    wbd, outd)
    nc.compile()
    _cache[key] = nc
    return nc


def _build_tile(nc, tc, xd, rbd, w1d, wcd, w2d, vpd, oad, owd, vbd, obd, wbd,
                outd):
    def pool(name, bufs, space="SBUF"):
        return tc.alloc_tile_pool(name=name, bufs=bufs, space=space)

    # ---- base pools: live for the whole program ----
    base_p = pool("base", 1)
    st2_p = pool("st2", 3)          # [128,512] staging (spills + outputs)
    ps_conv = pool("ps_conv", 4, space="PSUM")
    ps_misc = pool("ps_misc", 2, space="PSUM")
    ps_tr = pool("ps_tr", 2, space="PSUM")
    dram_p = pool("scratch", 1, space="DRAM")

    ident = base_p.tile([128, 128], F32)
    make_identity(nc, ident[:])
    ones1 = base_p.tile([1, 128], F32R)
    nc.vector.memset(ones1[:].bitcast(F32), 1.0)
    vb1 = base_p.tile([1, D], F32R)
    nc.sync.dma_start(vb1[:], vbd.ap())
    vbias = base_p.tile([128, D], F32)
    psb = ps_misc.tile([128, 512], F32, tag="psv", name="psb")
    nc.tensor.matmul(psb[:, :D], ones1[:], vb1[:], start=True, stop=True)
    nc.vector.tensor_copy(vbias[:], psb[:, :D])
    ob1 = base_p.tile([1, 96], F32R)
    nc.sync.dma_start(ob1[:], obd.ap())
    obias = base_p.tile([128, 96], F32)
    psb2 = ps_misc.tile([128, 512], F32, tag="psv", name="psb2")
    nc.tensor.matmul(psb2[:, :96], ones1[:], ob1[:], start=True, stop=True)
    nc.vector.tensor_copy(obias[:], psb2[:, :96])
    wbias = base_p.tile([128, 2], F32)
    nc.sync.dma_start(wbias[:], _ap(wbd, 0, [[1, 128], [128, 2]]))
    vproj = base_p.tile([128, 2, D], F32R)
    nc.sync.dma_start(vproj[:], _ap(vpd, 0, [[D, 128], [128 * D, 2], [1, D]]))
    offaw = base_p.tile([128, 2, 96], F32R)
    nc.sync.dma_start(offaw[:],
                      _ap(oad, 0, [[96, 128], [128 * 96, 2], [1, 96]]))
    outw = base_p.tile([128, 2, D], BF16)
    nc.sync.dma_start(outw[:], _ap(owd, 0, [[D, 128], [128 * D, 2], [1, D]]))
    offaw_n = base_p.tile([128, LT, 96], F32)
    rb = base_p.tile([128, LT, 2], F32)
    nc.sync.dma_start(rb[:], _ap(rbd, 0, [[2, 128], [256, LT], [1, 2]]))
    gxb = base_p.tile([128, LT, 2], F32)
    nc.scalar.activation(gxb[:], rb[:], ACTF.Copy, bias=-1.0, scale=64.0)

    bf_dram = dram_p.tile([8, 128, L], BF16)   # a,b,b1,b2 k-tiles for cv2
    v4 = [nc.dram_tensor(f"v4_{h}", [V4ROWS, 128], F32, kind="Internal")
          for h in range(NH)]
    idx_dram = dram_p.tile([NH * L * NP], I16)

    def spill_chunk(src_ap, slot_k, n):
        """cast a [128,8,64] f32(r) view to bf16 and store to bf_dram."""
        t = st2_p.tile([128, 512], BF16, tag="st2", name="spl")
        dst = _tap(t, 0, [[512, 128], [64, 8], [1, 64]])
        nc.vector.tensor_copy(dst, src_ap)
        nc.sync.dma_start(
            _tap(bf_dram, slot_k * 128 * L + n * 512, [[L, 128], [1, 512]]),
            t[:])

    def spill_map(src_view_fn, slot):
        """spill a 256-ch map (two [128, 64rows, 64] views) to bf_dram."""
        for k in range(2):
            v = src_view_fn(k)
            for n in range(NT):
                sub = AP(v.tensor, v.offset + (n * 8) * v.ap[1][0],
                         [[v.ap[0][0], 128], [v.ap[1][0], 8], [1, 64]])
                spill_chunk(sub, slot * 2 + k, n)

    # ================= scope 1: cv1 + bottlenecks + projections ==========
    s1_p = pool("s1", 1)      # xt
    wc_p = pool("wcp", 2)     # conv weight halves (9KB slots)
    big_p = pool("bigp", 2)   # 35KB slots: pads, b2, value (rotating)

    xt = s1_p.tile([128, 4, L], F32R, tag="xt")
    nc.sync.dma_start(xt[:], _ap(xd, 0, [[L, 128], [128 * L, 4], [1, L]]))
    w1 = wc_p.tile([128, 4, C1], F32R, tag="wc", name="w1")
    nc.sync.dma_start(w1[:], _ap(w1d, 0, [[C1, 128], [128 * C1, 4], [1, C1]]))

    b_pad = big_p.tile([128, 2, PADLEN], F32R, tag="big", name="b_pad")
    nc.vector.memset(b_pad[:].bitcast(F32), 0.0)

    for m in range(4):
        for n in range(NT):
            ps = ps_conv.tile([128, 512], F32, tag="conv_ps")
            for k in range(4):
                nc.tensor.matmul(
                    ps[:],
                    w1[:, k, m * 128:(m + 1) * 128],
                    xt[:, k, n * 512:(n + 1) * 512],
                    start=(k == 0), stop=(k == 3))
            if m < 2:
                # 'a' goes straight to DRAM as bf16 (k-tile slot m)
                t = st2_p.tile([128, 512], BF16, tag="st2", name="a_st")
                nc.scalar.activation(t[:], ps[:], ACT_MAIN)
                nc.sync.dma_start(
                    _tap(bf_dram, m * 128 * L + n * 512, [[L, 128], [1, 512]]),
                    t[:])
            else:
                # scatter 512 pixels = 8 rows of 64 into the padded layout
                row0 = n * 8
                dst = _tap(b_pad,
                           (m - 2) * PADLEN + DOFF + (row0 + 1) * PW + 1,
                           [[b_pad[:].ap[0][0], 128], [PW, 8], [1, 64]])
                src = _tap(ps, 0, [[ps[:].ap[0][0], 128], [64, 8], [1, 64]])
                nc.scalar.activation(dst, src, ACT_MAIN)

    # ---- bottleneck convs ----
    wc_tiles = {}

    def load_wc(ci, m):
        t = wc_p.tile([128, 9, 2, 128], F32R, tag="wc", name="wch")
        nc.sync.dma_start(
            t[:], _ap(wcd, ci * 9 * C * C + m * 128,
                      [[C, 128], [C * C, 9], [128 * C, 2], [1, 128]]))
        wc_tiles[(ci, m)] = t

    wcseq = [(ci, m) for ci in range(4) for m in range(2)]
    load_wc(0, 0)

    def conv3x3(src, ci, dst_fn, chunks=None):
        """src: padded [128,2,PADLEN] tile. dst_fn(m, pos, nsz, psum).
        pos/nsz index the 4224-long out region (padded idx DOFF+66+o)."""
        sst = src[:].ap[0][0]
        if chunks is None:
            chunks = [(i * 512, min(512, OUTREG - i * 512)) for i in range(9)]
        for m in range(2):
            wt = wc_tiles[(ci, m)]
            nxt = wcseq.index((ci, m)) + 1
            if nxt < len(wcseq) and wcseq[nxt] not in wc_tiles:
                load_wc(*wcseq[nxt])
            for pos, nsz in chunks:
                ps = ps_conv.tile([128, 512], F32, tag="conv_ps")
                i = 0
                for tap in range(9):
                    ty, tx = tap // 3, tap % 3
                    off = DOFF + pos + ty * PW + tx - 1
                    for k in range(2):
                        nc.tensor.matmul(
                            ps[:, :nsz],
                            wt[:, tap, k, :],
                            _tap(src, k * PADLEN + off,
                                 [[sst, 128], [1, nsz]]),
                            start=(i == 0), stop=(i == 17))
                        i += 1
                dst_fn(m, pos, nsz, ps)

    def pad_writer(dst):
        def f(m, pos, nsz, ps):
            nc.scalar.activation(
                _tap(dst, m * PADLEN + DOFF + PW + pos,
                     [[dst[:].ap[0][0], 128], [1, nsz]]),
                ps[:, :nsz], ACT_MAIN)
        return f

    def zero_padcols(t):
        nc.vector.memset(
            _tap(t, DOFF + PW, [[t[:].ap[0][0], 128], [PADLEN, 2], [PW, 64],
                                [65, 2]]).bitcast(F32), 0.0)

    def padded_view(t, k):
        return _tap(t, k * PADLEN + DOFF + PW + 1,
                    [[t[:].ap[0][0], 128], [PW, 64], [1, 64]])

    mid = big_p.tile([128, 2, PADLEN], F32R, tag="big", name="mid")
    nc.vector.memset(mid[:].bitcast(F32), 0.0)
    conv3x3(b_pad, 0, pad_writer(mid))
    zero_padcols(mid)
    spill_map(lambda k: padded_view(b_pad, k), 1)

    b1_pad = big_p.tile([128, 2, PADLEN], F32R, tag="big", name="b1_pad")
    nc.vector.memset(b1_pad[:].bitcast(F32), 0.0)
    conv3x3(mid, 1, pad_writer(b1_pad))
    zero_padcols(b1_pad)

    mid2 = big_p.tile([128, 2, PADLEN], F32R, tag="big", name="mid2")
    nc.vector.memset(mid2[:].bitcast(F32), 0.0)
    conv3x3(b1_pad, 2, pad_writer(mid2))
    zero_padcols(mid2)
    spill_map(lambda k: padded_view(b1_pad, k), 2)

    b2 = big_p.tile([128, 2, PADLEN], F32R, tag="big", name="b2")

    def b2_writer(m, pos, nsz, ps):
        row0, nrow = pos // PW, nsz // PW
        dst = _tap(b2, m * PADLEN + row0 * 64,
                   [[b2[:].ap[0][0], 128], [64, nrow], [1, 64]])
        src = _tap(ps, 1, [[ps[:].ap[0][0], 128], [PW, nrow], [1, 64]])
        nc.scalar.activation(dst, src, ACT_MAIN)

    rowchunks = [(rc * 4 * PW, 4 * PW) for rc in range(16)]  # 264 each
    conv3x3(mid2, 3, b2_writer, chunks=rowchunks)

    if PH < 2:
        big_p.release(); wc_p.release(); s1_p.release()
        dram_p.release(); ps_tr.release(); ps_misc.release()
        ps_conv.release(); st2_p.release(); base_p.release()
        return
    # ---- projections ----
    value = big_p.tile([128, 2, PADLEN], F32, tag="big", name="value")
    vst = value[:].ap[0][0]

    for lt in range(LT):
        psv = ps_misc.tile([128, 512], F32, tag="psv")
        for k in range(2):
            nc.tensor.matmul(psv[:, :D],
                             _tap(b2, k * PADLEN + lt * 128,
                                  [[b2[:].ap[0][0], 128], [1, 128]]),
                             vproj[:, k, :],
                             start=(k == 0), stop=(k == 1))
        nc.vector.tensor_tensor(_tap(value, lt * D, [[vst, 128], [1, D]]),
                                psv[:, :D], vbias[:], ALU.add)
        pso = ps_misc.tile([128, 512], F32, tag="psv")
        for k in range(2):
            nc.tensor.matmul(pso[:, :96],
                             _tap(b2, k * PADLEN + lt * 128,
                                  [[b2[:].ap[0][0], 128], [1, 128]]),
                             offaw[:, k, :],
                             start=(k == 0), stop=(k == 1))
        nc.vector.tensor_tensor(offaw_n[:, lt, :], pso[:, :96], obias[:],
                                ALU.add)
    spill_map(lambda k: _tap(b2, k * PADLEN,
                             [[b2[:].ap[0][0], 128], [64, 64], [1, 64]]), 3)

    # V4: per head h, row (66+i) = [V[i], V[i+1], V[i+64], V[i+65]]
    for hh in range(NH):
        for si, dlt in enumerate([0, 1, 64, 65]):
            src = _tap(value, hh * DH, [[vst, 128], [D, LT], [1, DH]])
            dst = _ap(v4[hh], (66 - dlt) * 128 + si * DH,
                      [[128, 128], [128 * 128, LT], [1, DH]])
            nc.sync.dma_start(dst, src)

    big_p.release()
    wc_p.release()
    s1_p.release()

    if PH < 3:
        dram_p.release(); ps_tr.release(); ps_misc.release()
        ps_conv.release(); st2_p.release(); base_p.release()
        return

    # ================= scope 2: coords + gather + attn + cv2 =============
    ctmp_p = pool("ctmp", 1)
    coef_p = pool("coefp", 1)
    pre_p = pool("prep", 1)
    apt_p = pool("aptp", 2)
    attnT_p = pool("attnTp", 1)
    gat_p = pool("gatp", 2)
    kst_p = pool("kst", 3)
    w2_p = pool("w2p", 1)

    coef = coef_p.tile([128, NH, LT, NP, 4], BF16, tag="coef")
    cst = coef[:].ap[0][0]
    idx16 = coef_p.tile([128, NH, LT, NP], I16, tag="idx16")
    ist = idx16[:].ap[0][0]
    idx_wr = coef_p.tile([128, NH * 1024], I16, tag="idx_wr")

    # ---- sampling coordinates, in two l-halves ----
    SH = [128, HLT, 32]
    ost = offaw_n[:].ap[0][0]

    def lhc(t):
        """[128, HLT, 32combo] tile viewed as [part, lt, h, p]."""
        return _tap(t, 0, [[t[:].ap[0][0], 128], [32, HLT], [NP, NH],
                           [1, NP]])

    cp = ctmp_p
    for lh in range(2):
        lt0 = lh * HLT

        def off_view(xy):
            return _tap(offaw_n, lt0 * 96 + xy,
                        [[ost, 128], [96, HLT], [2, 32]])

        def axis_weights(xy, lim):
            g = cp.tile(SH, F32, tag="g")
            gb = _tap(gxb, lt0 * 2 + xy,
                      [[gxb[:].ap[0][0], 128], [2, HLT], [0, 32]])
            nc.vector.tensor_tensor(g[:], off_view(xy), gb, ALU.add)
            # g holds g_true-0.5 (gxb bias -1.0 = grid's -0.5 plus -0.5
            # for round->floor). x0 = round(g) = floor(g_true) via the fp32
            # magic constant (2^23*1.5, representable; at g_true exactly
            # integer the half-even tie gives floor or floor-1, both of
            # which produce identical interpolation).
            x0 = cp.tile(SH, F32, tag="x0")
            nc.vector.tensor_scalar(x0[:], g[:], 12582912.0, 12582912.0,
                                    ALU.add, ALU.subtract)
            fr = cp.tile(SH, F32, tag="t1", name="fr")
            nc.vector.tensor_tensor(fr[:], g[:], x0[:], ALU.subtract)
            wfrac = cp.tile(SH, F32, tag="wf")
            nc.vector.tensor_scalar(wfrac[:], fr[:], 0.5, None, ALU.add)
            wcmp = cp.tile(SH, F32, tag="wcm")
            nc.vector.tensor_scalar(wcmp[:], fr[:], -1.0, 0.5, ALU.mult,
                                    ALU.add)
            bx = cp.tile(SH, F32, tag=f"bx{xy}")
            nc.vector.tensor_scalar(bx[:], x0[:], 0.0, float(lim), ALU.max,
                                    ALU.min)
            d = cp.tile(SH, F32, tag="d")
            nc.vector.tensor_tensor(d[:], bx[:], x0[:], ALU.subtract)
            e0 = cp.tile(SH, F32, tag="e0")
            nc.vector.tensor_scalar(e0[:], d[:], 0.0, None, ALU.is_equal)
            em = cp.tile(SH, F32, tag="em")
            nc.vector.tensor_scalar(em[:], d[:], 1.0, None, ALU.is_equal)
            ep = cp.tile(SH, F32, tag="ep")
            nc.vector.tensor_scalar(ep[:], d[:], -1.0, None, ALU.is_equal)
            t1 = cp.tile(SH, F32, tag="t1")
            s0 = cp.tile(SH, F32, tag=f"s0{xy}")
            nc.vector.tensor_tensor(t1[:], e0[:], wcmp[:], ALU.mult)
            nc.vector.tensor_tensor(s0[:], em[:], wfrac[:], ALU.mult)
            nc.vector.tensor_tensor(s0[:], s0[:], t1[:], ALU.add)
            s1 = cp.tile(SH, F32, tag=f"s1{xy}")
            nc.vector.tensor_tensor(t1[:], e0[:], wfrac[:], ALU.mult)
            nc.vector.tensor_tensor(s1[:], ep[:], wcmp[:], ALU.mult)
            nc.vector.tensor_tensor(s1[:], s1[:], t1[:], ALU.add)
            return s0, s1, bx

        sx0, sx1, bxx = axis_weights(0, W - 2)
        sy0, sy1, bxy = axis_weights(1, H - 2)

        idxf = cp.tile(SH, F32, tag="g", name="idxf")
        nc.vector.tensor_scalar(idxf[:], bxy[:], float(W), 66.0, ALU.mult,
                                ALU.add)
        nc.vector.tensor_tensor(idxf[:], idxf[:], bxx[:], ALU.add)
        idx_dst = _tap(idx16, lt0 * NP,
                       [[ist, 128], [NP, HLT], [LT * NP, NH], [1, NP]])
        nc.vector.tensor_copy(idx_dst, lhc(idxf))

        # softmax over p
        aw4 = _tap(offaw_n, lt0 * 96 + 64,
                   [[ost, 128], [96, HLT], [4, NH], [1, NP]])
        mx = cp.tile([128, HLT, NH], F32, tag="em", name="mx")
        nc.vector.tensor_reduce(mx[:], aw4, AX.X, ALU.max)
        mxb = _tap(mx, 0, [[mx[:].ap[0][0], 128], [NH, HLT], [1, NH],
                           [0, NP]])
        z = cp.tile(SH, F32, tag="x0", name="z")
        zv = _tap(z, 0, [[z[:].ap[0][0], 128], [32, HLT], [4, NH], [1, NP]])
        nc.vector.tensor_tensor(zv, aw4, mxb, ALU.subtract)
        ez = cp.tile(SH, F32, tag="d", name="ez")
        nc.scalar.activation(ez[:], z[:], ACTF.Exp)
        ezv = _tap(ez, 0, [[ez[:].ap[0][0], 128], [32, HLT], [4, NH],
                           [1, NP]])
        ssum = cp.tile([128, HLT, NH], F32, tag="ep", name="ssum")
        nc.vector.tensor_reduce(ssum[:], ezv, AX.X, ALU.add)
        rs = cp.tile([128, HLT, NH], F32, tag="t1", name="rs")
        nc.vector.reciprocal(rs[:], ssum[:])
        rsb = _tap(rs, 0, [[rs[:].ap[0][0], 128], [NH, HLT], [1, NH],
                           [0, NP]])
        Aw = cp.tile(SH, F32, tag="e0", name="Aw")
        Av = _tap(Aw, 0, [[Aw[:].ap[0][0], 128], [32, HLT], [4, NH], [1, NP]])
        nc.vector.tensor_tensor(Av, ezv, rsb, ALU.mult)

        tprod = cp.tile(SH, F32, tag="wf", name="tprod")
        for slot, (sa, sb) in enumerate([(sx0, sy0), (sx1, sy0), (sx0, sy1),
                                         (sx1, sy1)]):
            nc.vector.tensor_tensor(tprod[:], sa[:], sb[:], ALU.mult)
            cdst = AP(coef[:].tensor, coef[:].offset + lt0 * NP * 4 + slot,
                      [[cst, 128], [NP * 4, HLT], [LT * NP * 4, NH], [4, NP]])
            nc.vector.tensor_tensor(cdst, lhc(tprod), lhc(Aw), ALU.mult)

    # ---- idx shuffle via DRAM into 16-partition-wrapped layout ----
    nc.sync.dma_start(
        _tap(idx_dram, 0, [[1, 128], [L * NP, NH], [512, LT], [128, NP]]),
        idx16[:])
    for grp in range(8):
        nc.sync.dma_start(idx_wr[16 * grp:16 * (grp + 1), :],
                          _tap(idx_dram, 0, [[1, 16], [16, NH * 1024]]))

    if PH < 4:
        w2_p.release(); kst_p.release(); gat_p.release()
        attnT_p.release(); apt_p.release(); pre_p.release()
        coef_p.release(); ctmp_p.release(); dram_p.release()
        ps_tr.release(); ps_misc.release(); ps_conv.release()
        st2_p.release(); base_p.release()
        return

    # ---- gather + weighted reduce + transpose ----
    attn_preT = []
    for hg in range(2):
        pre = pre_p.tile([128, LT, 4, DH], F32, tag="pre")
        prest = pre[:].ap[0][0]
        for hi in range(4):
            h = hg * 4 + hi
            for q in range(4):      # quarter-head chunks: 8 l-tiles each
                g = gat_p.tile([128, 32, 128], F32, tag="gat")
                idxs = idx_wr[:, h * 1024 + q * 256:h * 1024 + (q + 1) * 256]
                nc.gpsimd.dma_gather(
                    g[:],
                    _ap(v4[h], 0, [[128, 4097], [1, 128]]),
                    idxs, 4096, 4096, 128, single_packet=False)
                gst = g[:].ap[0][0]
                gv = _tap(g, 0, [[gst, 128], [512, 8], [128, NP], [32, 4],
                                 [1, DH]])
                cch = AP(coef[:].tensor,
                         coef[:].offset + h * LT * NP * 4 + q * 8 * NP * 4,
                         [[cst, 128], [16, 8], [4, NP], [1, 4], [0, DH]])
                nc.vector.tensor_tensor(gv, gv, cch, ALU.mult)
                gr = _tap(g, 0, [[gst, 128], [512, 8], [1, DH], [32, 16]])
                nc.vector.tensor_reduce(
                    _tap(pre, (q * 8) * 4 * DH + hi * DH,
                         [[prest, 128], [4 * DH, 8], [1, DH]]),
                    gr, AX.X, ALU.add)
        # transpose [128 l_lo, 128 (4 heads x 32 dh)] per l-tile -> bf16
        apt = apt_p.tile([128, L], BF16, tag="apT", name="apt")
        attn_preT.append(apt)
        for lt in range(LT):
            pst = ps_tr.tile([128, 128], F32, tag="pst")
            nc.tensor.transpose(pst[:], pre[:, lt, :, :], ident[:])
            nc.vector.tensor_copy(apt[:, lt * 128:(lt + 1) * 128], pst[:])

    if PH < 5:
        w2_p.release(); kst_p.release(); gat_p.release()
        attnT_p.release(); apt_p.release(); pre_p.release()
        coef_p.release(); ctmp_p.release(); dram_p.release()
        ps_tr.release(); ps_misc.release(); ps_conv.release()
        st2_p.release(); base_p.release()
        return

    # ---- attn out-projection (bf16, +out_b) ----
    attnT_bf = attnT_p.tile([128, 2, L], BF16, tag="attnT")
    for mg in range(2):
        for n in range(NT):
            ps = ps_misc.tile([128, 512], F32, tag="psv")
            for k in range(2):
                nc.tensor.matmul(
                    ps[:],
                    outw[:, k, mg * 128:(mg + 1) * 128],
                    attn_preT[k][:, n * 512:(n + 1) * 512],
                    start=(k == 0), stop=(k == 1))
            nc.scalar.activation(attnT_bf[:, mg, n * 512:(n + 1) * 512],
                                 ps[:], ACTF.Identity,
                                 bias=wbias[:, mg:mg + 1])

    # ---- cv2 ----
    w2a = w2_p.tile([128, 5, C2], BF16, tag="w2a")
    nc.sync.dma_start(w2a[:], _ap(w2d, 0, [[C2, 128], [128 * C2, 5], [1, C2]]))
    w2b = w2_p.tile([128, 5, C2], BF16, tag="w2b")
    nc.sync.dma_start(w2b[:],
                      _ap(w2d, 5 * 128 * C2, [[C2, 128], [128 * C2, 5],
                                              [1, C2]]))

    for n in range(NT):
        ktiles = []
        for kk in range(8):
            t = kst_p.tile([128, 512], BF16, tag="kstream")
            nc.sync.dma_start(
                t[:], _tap(bf_dram, kk * 128 * L + n * 512,
                           [[L, 128], [1, 512]]))
            ktiles.append(t)
        for m in range(4):
            ps = ps_conv.tile([128, 512], F32, tag="conv_ps")
            for k in range(10):
                rhs = (ktiles[k][:] if k < 8
                       else attnT_bf[:, k - 8, n * 512:(n + 1) * 512])
                wt = w2a if k < 5 else w2b
                nc.tensor.matmul(ps[:], wt[:, k % 5, m * 128:(m + 1) * 128],
                                 rhs, start=(k == 0), stop=(k == 9))
            o = st2_p.tile([128, 512], F32, tag="st2", name="o")
            nc.scalar.activation(o[:], ps[:], ACT_MAIN)
            nc.sync.dma_start(
                _ap(outd, m * 128 * L + n * 512, [[L, 128], [1, 512]]), o[:])

    w2_p.release()
    kst_p.release()
    gat_p.release()
    attnT_p.release()
    apt_p.release()
    pre_p.release()
    coef_p.release()
    ctmp_p.release()
    dram_p.release()
    ps_tr.release()
    ps_misc.release()
    ps_conv.release()
    st2_p.release()
    base_p.release()


def host_prep(inputs):
    import ml_dtypes
    x = np.asarray(inputs["x"], np.float32).reshape(B, C1, L)
    rb = np.asarray(inputs["refer_bbox"], np.float32).reshape(B, L, 2)
    w1t = np.ascontiguousarray(
        np.asarray(inputs["cv1_w"], np.float32)[:, :, 0, 0].T)
    wc = np.ascontiguousarray(np.stack([
        np.asarray(inputs[k], np.float32).transpose(2, 3, 1, 0).reshape(
            9, C, C)
        for k in ["m0_cv1_w", "m0_cv2_w", "m1_cv1_w", "m1_cv2_w"]]))
    w2t = np.ascontiguousarray(
        np.asarray(inputs["cv2_w"], np.float32)[:, :, 0, 0].T).astype(
            ml_dtypes.bfloat16)
    out_w = np.ascontiguousarray(
        np.asarray(inputs["out_w"], np.float32)).astype(ml_dtypes.bfloat16)
    shared = {
        "w1t": w1t, "wc": wc, "w2t": w2t, "out_w": out_w,
        "vproj_w": np.ascontiguousarray(
            np.asarray(inputs["vproj_w"], np.float32)),
        "offaw_w": np.ascontiguousarray(np.concatenate(
            [np.asarray(inputs["off_w"], np.float32),
             np.asarray(inputs["aw_w"], np.float32)], axis=1)),
        "vproj_b": np.asarray(inputs["vproj_b"], np.float32).reshape(1, D),
        "offaw_b": np.ascontiguousarray(np.concatenate(
            [np.asarray(inputs["off_b"], np.float32),
             np.asarray(inputs["aw_b"], np.float32)]).reshape(1, 96)),
        "out_b": np.asarray(inputs["out_b"], np.float32).reshape(D, 1),
    }
    in_maps = []
    for c in range(B):
        m = dict(shared)
        m["x"] = np.ascontiguousarray(x[c])
        m["refer"] = np.ascontiguousarray(rb[c])
        in_maps.append(m)
    return in_maps


def kernel(**inputs):
    nc = build(B)
    in_maps = host_prep(inputs)
    res = run_bass_kernel_spmd(nc, in_maps, core_ids=list(range(B)))
    out = np.stack([res.results[c]["out"].reshape(C2, H, W) for c in range(B)])
    return out.astype(np.float32)


if __name__ == "__main__":
    build()
    print("build ok")



# revision 9
# speedup vs baseline: 3.2887x; 3.2887x over previous
"""Trainium2 Bass kernel for C2f-with-DeformableAttention block.

Sharding: data-parallel over batch (8 images -> 8 NeuronCores), weights
replicated, no collectives. Each core runs the full block for one image:
  cv1 (1x1) -> split a/b -> 2x Bottleneck(3x3+3x3) -> msdeform attn
  -> concat(a,b,b1,b2,attn) -> cv2 (1x1), SiLU after every conv.

Per-core layouts:
  feature maps: channel-major [C partitions, H*W free]; 3x3-conv inputs are
  zero-padded [C, 66*66] so the 9 taps are contiguous shifted reads feeding
  PSUM-accumulated matmuls.
  deformable sampling: a per-head V4 table in DRAM packs the 4 bilinear
  corner pixels per base index into 512B rows, fetched by SWDGE dma_gather;
  bilinear + softmax attention weights fold into 16 coefficients per sample
  applied with one multiply + one strided reduce on VectorE. Border clipping
  is folded into the coefficients (base index clamped to [0,62]^2, weights
  remapped/zeroed), matching grid_sample(align_corners=False) + masking.
Matmuls run float32r (full-rate fp32 PE mode, fp32 PSUM accumulation); the
concat/cv2 and attention-output paths are bf16. SBUF pressure is managed
with two sequential pool scopes (convs, attention) plus tag-based slot
rotation for the large maps.
"""

import os
import sys

sys.path.insert(0, "/opt/trn_rl_repo")

import numpy as np

import concourse.bass as bass
import concourse.tile as tile
from concourse import bacc, mybir
from concourse.bass import AP
from concourse.bass_utils import run_bass_kernel_spmd
from concourse.masks import make_identity

F32 = mybir.dt.float32
F32R = mybir.dt.float32r
BF16 = mybir.dt.bfloat16
I16 = mybir.dt.int16
ALU = mybir.AluOpType
ACTF = mybir.ActivationFunctionType
AX = mybir.AxisListType

B, C1, C2 = 8, 512, 512
C = 256
D = 256
NH, NP = 8, 4
H = W = 64
L = H * W            # 4096
DH = D // NH         # 32
PW = W + 2           # 66
DOFF = 1             # leading pad element so tap offset -1 stays in-tile
PADLEN = PW * 66 + 16   # per-channel padded map length (+DOFF+tail slack)
OUTREG = 64 * PW     # 4224: contiguous output region = rows 1..64 (all cols)
V4ROWS = 4168        # 66 front pad + 4096 rows + tail
LT = L // 128        # 32
HLT = LT // 2        # 16 (coords run in two l-halves)
NT = L // 512        # 8

SIM_ACT = os.environ.get("BASS_KERNEL_SIM_ACT", "") == "sigmoid"
PH = int(os.environ.get("BASS_KERNEL_PHASES", "9"))
ACT_MAIN = ACTF.Sigmoid if SIM_ACT else ACTF.Silu

_cache = {}


def _ap(t, offset, dims):
    """AP into a DRAM tensor handle at element offset."""
    return AP(t.ap().tensor, offset, dims)


def _tap(tile_, offset, dims):
    """AP into an SBUF/DRAM tile at element offset from tile base."""
    a = tile_[:]
    return AP(a.tensor, a.offset + offset, dims)


def build(n_cores=8):
    key = ("nc", SIM_ACT, PH)
    if key in _cache:
        return _cache[key]
    nc = bacc.Bacc("TRN2", target_bir_lowering=False, debug=False,
                   num_devices=n_cores, num_swdge_queues=4)

    xd = nc.dram_tensor("x", [C1, L], F32R, kind="ExternalInput")
    rbd = nc.dram_tensor("refer", [L, 2], F32, kind="ExternalInput")
    w1d = nc.dram_tensor("w1t", [C1, C1], F32R, kind="ExternalInput")
    wcd = nc.dram_tensor("wc", [4, 9, C, C], F32R, kind="ExternalInput")
    w2d = nc.dram_tensor("w2t", [5 * C, C2], BF16, kind="ExternalInput")
    vpd = nc.dram_tensor("vproj_w", [D, D], F32R, kind="ExternalInput")
    oad = nc.dram_tensor("offaw_w", [D, 96], F32R, kind="ExternalInput")
    owd = nc.dram_tensor("out_w", [D, D], BF16, kind="ExternalInput")
    vbd = nc.dram_tensor("vproj_b", [1, D], F32R, kind="ExternalInput")
    obd = nc.dram_tensor("offaw_b", [1, 96], F32R, kind="ExternalInput")
    wbd = nc.dram_tensor("out_b", [D, 1], F32, kind="ExternalInput")
    outd = nc.dram_tensor("out", [C2, L], F32, kind="ExternalOutput")

    with tile.TileContext(nc) as tc:
        _build_tile(nc, tc, xd, rbd, w1d, wcd, w2d, vpd, oad, owd, vbd, obd,
                    wbd, outd)
    nc.compile()
    _cache[key] = nc
    return nc


def _build_tile(nc, tc, xd, rbd, w1d, wcd, w2d, vpd, oad, owd, vbd, obd, wbd,
                outd):
    def pool(name, bufs, space="SBUF"):
        return tc.alloc_tile_pool(name=name, bufs=bufs, space=space)

    # ---- base pools: live for the whole program ----
    base_p = pool("base", 1)
    st2_p = pool("st2", 3)          # [128,512] staging (spills + outputs)
    ps_conv = pool("ps_conv", 4, space="PSUM")
    ps_misc = pool("ps_misc", 2, space="PSUM")
    ps_tr = pool("ps_tr", 2, space="PSUM")
    dram_p = pool("scratch", 1, space="DRAM")

    ident = base_p.tile([128, 128], F32)
    make_identity(nc, ident[:])
    ones1 = base_p.tile([1, 128], F32R)
    nc.vector.memset(ones1[:].bitcast(F32), 1.0)
    vb1 = base_p.tile([1, D], F32R)
    nc.sync.dma_start(vb1[:], vbd.ap())
    vbias = base_p.tile([128, D], F32)
    psb = ps_misc.tile([128, 512], F32, tag="psv", name="psb")
    nc.tensor.matmul(psb[:, :D], ones1[:], vb1[:], start=True, stop=True)
    nc.vector.tensor_copy(vbias[:], psb[:, :D])
    ob1 = base_p.tile([1, 96], F32R)
    nc.sync.dma_start(ob1[:], obd.ap())
    obias = base_p.tile([128, 96], F32)
    psb2 = ps_misc.tile([128, 512], F32, tag="psv", name="psb2")
    nc.tensor.matmul(psb2[:, :96], ones1[:], ob1[:], start=True, stop=True)
    nc.vector.tensor_copy(obias[:], psb2[:, :96])
    wbias = base_p.tile([128, 2], F32)
    nc.sync.dma_start(wbias[:], _ap(wbd, 0, [[1, 128], [128, 2]]))
    vproj = base_p.tile([128, 2, D], F32R)
    nc.sync.dma_start(vproj[:], _ap(vpd, 0, [[D, 128], [128 * D, 2], [1, D]]))
    offaw = base_p.tile([128, 2, 96], F32R)
    nc.sync.dma_start(offaw[:],
                      _ap(oad, 0, [[96, 128], [128 * 96, 2], [1, 96]]))
    outw = base_p.tile([128, 2, D], BF16)
    nc.sync.dma_start(outw[:], _ap(owd, 0, [[D, 128], [128 * D, 2], [1, D]]))
    offaw_n = base_p.tile([128, LT, 96], F32)
    rb = base_p.tile([128, LT, 2], F32)
    nc.sync.dma_start(rb[:], _ap(rbd, 0, [[2, 128], [256, LT], [1, 2]]))
    gxb = base_p.tile([128, LT, 2], F32)
    nc.scalar.activation(gxb[:], rb[:], ACTF.Copy, bias=-1.0, scale=64.0)

    bf_dram = dram_p.tile([8, 128, L], BF16)   # a,b,b1,b2 k-tiles for cv2
    v4 = [nc.dram_tensor(f"v4_{h}", [V4ROWS, 128], BF16, kind="Internal")
          for h in range(NH)]
    # idx staging: [8192 rows, 128 cols] i16; read back via xbar transpose
    idx_dram = dram_p.tile([NH * 128 * 1024], I16)

    def spill_chunk(src_ap, slot_k, n):
        """cast a [128,8,64] f32(r) view to bf16 and store to bf_dram."""
        t = st2_p.tile([128, 512], BF16, tag="st2", name="spl")
        dst = _tap(t, 0, [[512, 128], [64, 8], [1, 64]])
        nc.vector.tensor_copy(dst, src_ap)
        nc.sync.dma_start(
            _tap(bf_dram, slot_k * 128 * L + n * 512, [[L, 128], [1, 512]]),
            t[:])

    def spill_map(src_view_fn, slot):
        """spill a 256-ch map (two [128, 64rows, 64] views) to bf_dram."""
        for k in range(2):
            v = src_view_fn(k)
            for n in range(NT):
                sub = AP(v.tensor, v.offset + (n * 8) * v.ap[1][0],
                         [[v.ap[0][0], 128], [v.ap[1][0], 8], [1, 64]])
                spill_chunk(sub, slot * 2 + k, n)

    # ================= scope 1: cv1 + bottlenecks + projections ==========
    s1_p = pool("s1", 1)      # xt
    wc_p = pool("wcp", 2)     # conv weight halves (9KB slots)
    big_p = pool("bigp", 2)   # 35KB slots: pads, b2, value (rotating)

    xt = s1_p.tile([128, 4, L], F32R, tag="xt")
    for n in range(NT):
        nc.sync.dma_start(
            _tap(xt, n * 512, [[4 * L, 128], [L, 4], [1, 512]]),
            _ap(xd, n * 512, [[L, 128], [128 * L, 4], [1, 512]]))
    w1 = wc_p.tile([128, 4, C1], F32R, tag="wc", name="w1")
    nc.sync.dma_start(w1[:], _ap(w1d, 0, [[C1, 128], [128 * C1, 4], [1, C1]]))

    b_pad = big_p.tile([128, 2, PADLEN], F32R, tag="big", name="b_pad")
    nc.vector.memset(b_pad[:].bitcast(F32), 0.0)

    for m in range(4):
        for n in range(NT):
            ps = ps_conv.tile([128, 512], F32, tag="conv_ps")
            for k in range(4):
                nc.tensor.matmul(
                    ps[:],
                    w1[:, k, m * 128:(m + 1) * 128],
                    xt[:, k, n * 512:(n + 1) * 512],
                    start=(k == 0), stop=(k == 3))
            if m < 2:
                # 'a' goes straight to DRAM as bf16 (k-tile slot m)
                t = st2_p.tile([128, 512], BF16, tag="st2", name="a_st")
                nc.scalar.activation(t[:], ps[:], ACT_MAIN)
                nc.sync.dma_start(
                    _tap(bf_dram, m * 128 * L + n * 512, [[L, 128], [1, 512]]),
                    t[:])
            else:
                # scatter 512 pixels = 8 rows of 64 into the padded layout
                row0 = n * 8
                dst = _tap(b_pad,
                           (m - 2) * PADLEN + DOFF + (row0 + 1) * PW + 1,
                           [[b_pad[:].ap[0][0], 128], [PW, 8], [1, 64]])
                src = _tap(ps, 0, [[ps[:].ap[0][0], 128], [64, 8], [1, 64]])
                nc.scalar.activation(dst, src, ACT_MAIN)

    # ---- bottleneck convs ----
    wc_tiles = {}

    def load_wc(ci, m):
        t = wc_p.tile([128, 9, 2, 128], F32R, tag="wc", name="wch")
        nc.sync.dma_start(
            t[:], _ap(wcd, ci * 9 * C * C + m * 128,
                      [[C, 128], [C * C, 9], [128 * C, 2], [1, 128]]))
        wc_tiles[(ci, m)] = t

    wcseq = [(ci, m) for ci in range(4) for m in range(2)]
    load_wc(0, 0)

    def conv3x3(src, ci, dst_fn, chunks=None):
        """src: padded [128,2,PADLEN] tile. dst_fn(m, pos, nsz, psum).
        pos/nsz index the 4224-long out region (padded idx DOFF+66+o)."""
        sst = src[:].ap[0][0]
        if chunks is None:
            chunks = [(i * 512, min(512, OUTREG - i * 512)) for i in range(9)]
        for m in range(2):
            wt = wc_tiles[(ci, m)]
            nxt = wcseq.index((ci, m)) + 1
            if nxt < len(wcseq) and wcseq[nxt] not in wc_tiles:
                load_wc(*wcseq[nxt])
            for pos, nsz in chunks:
                ps = ps_conv.tile([128, 512], F32, tag="conv_ps")
                i = 0
                for tap in range(9):
                    ty, tx = tap // 3, tap % 3
                    off = DOFF + pos + ty * PW + tx - 1
                    for k in range(2):
                        nc.tensor.matmul(
                            ps[:, :nsz],
                            wt[:, tap, k, :],
                            _tap(src, k * PADLEN + off,
                                 [[sst, 128], [1, nsz]]),
                            start=(i == 0), stop=(i == 17))
                        i += 1
                dst_fn(m, pos, nsz, ps)

    def pad_writer(dst):
        def f(m, pos, nsz, ps):
            nc.scalar.activation(
                _tap(dst, m * PADLEN + DOFF + PW + pos,
                     [[dst[:].ap[0][0], 128], [1, nsz]]),
                ps[:, :nsz], ACT_MAIN)
        return f

    def zero_padcols(t):
        nc.vector.memset(
            _tap(t, DOFF + PW, [[t[:].ap[0][0], 128], [PADLEN, 2], [PW, 64],
                                [65, 2]]).bitcast(F32), 0.0)

    def padded_view(t, k):
        return _tap(t, k * PADLEN + DOFF + PW + 1,
                    [[t[:].ap[0][0], 128], [PW, 64], [1, 64]])

    mid = big_p.tile([128, 2, PADLEN], F32R, tag="big", name="mid")
    nc.vector.memset(mid[:].bitcast(F32), 0.0)
    conv3x3(b_pad, 0, pad_writer(mid))
    zero_padcols(mid)
    spill_map(lambda k: padded_view(b_pad, k), 1)

    b1_pad = big_p.tile([128, 2, PADLEN], F32R, tag="big", name="b1_pad")
    nc.vector.memset(b1_pad[:].bitcast(F32), 0.0)
    conv3x3(mid, 1, pad_writer(b1_pad))
    zero_padcols(b1_pad)

    mid2 = big_p.tile([128, 2, PADLEN], F32R, tag="big", name="mid2")
    nc.vector.memset(mid2[:].bitcast(F32), 0.0)
    conv3x3(b1_pad, 2, pad_writer(mid2))
    zero_padcols(mid2)
    spill_map(lambda k: padded_view(b1_pad, k), 2)

    b2 = big_p.tile([128, 2, PADLEN], F32R, tag="big", name="b2")

    def b2_writer(m, pos, nsz, ps):
        row0, nrow = pos // PW, nsz // PW
        dst = _tap(b2, m * PADLEN + row0 * 64,
                   [[b2[:].ap[0][0], 128], [64, nrow], [1, 64]])
        src = _tap(ps, 1, [[ps[:].ap[0][0], 128], [PW, nrow], [1, 64]])
        nc.scalar.activation(dst, src, ACT_MAIN)

    rowchunks = [(rc * 4 * PW, 4 * PW) for rc in range(16)]  # 264 each
    conv3x3(mid2, 3, b2_writer, chunks=rowchunks)

    if PH < 2:
        big_p.release(); wc_p.release(); s1_p.release()
        dram_p.release(); ps_tr.release(); ps_misc.release()
        ps_conv.release(); st2_p.release(); base_p.release()
        return
    # ---- projections ----
    value = big_p.tile([128, 2, PADLEN], BF16, tag="big", name="value")
    vst = value[:].ap[0][0]

    for lt in range(LT):
        psv = ps_misc.tile([128, 512], F32, tag="psv")
        for k in range(2):
            nc.tensor.matmul(psv[:, :D],
                             _tap(b2, k * PADLEN + lt * 128,
                                  [[b2[:].ap[0][0], 128], [1, 128]]),
                             vproj[:, k, :],
                             start=(k == 0), stop=(k == 1))
        nc.vector.tensor_tensor(_tap(value, lt * D, [[vst, 128], [1, D]]),
                                psv[:, :D], vbias[:], ALU.add)
        pso = ps_misc.tile([128, 512], F32, tag="psv")
        for k in range(2):
            nc.tensor.matmul(pso[:, :96],
                             _tap(b2, k * PADLEN + lt * 128,
                                  [[b2[:].ap[0][0], 128], [1, 128]]),
                             offaw[:, k, :],
                             start=(k == 0), stop=(k == 1))
        nc.vector.tensor_tensor(offaw_n[:, lt, :], pso[:, :96], obias[:],
                                ALU.add)
    spill_map(lambda k: _tap(b2, k * PADLEN,
                             [[b2[:].ap[0][0], 128], [64, 64], [1, 64]]), 3)

    # V4: per head h, row (66+i) = [V[i], V[i+1], V[i+64], V[i+65]]
    for hh in range(NH):
        for si, dlt in enumerate([0, 1, 64, 65]):
            src = _tap(value, hh * DH, [[vst, 128], [D, LT], [1, DH]])
            dst = _ap(v4[hh], (66 - dlt) * 128 + si * DH,
                      [[128, 128], [128 * 128, LT], [1, DH]])
            nc.sync.dma_start(dst, src)

    big_p.release()
    wc_p.release()
    s1_p.release()

    if PH < 3:
        dram_p.release(); ps_tr.release(); ps_misc.release()
        ps_conv.release(); st2_p.release(); base_p.release()
        return

    # ================= scope 2: coords + gather + attn + cv2 =============
    ctmp_p = pool("ctmp", 1)
    coef_p = pool("coefp", 1)
    pre_p = pool("prep", 1)
    apt_p = pool("aptp", 2)
    attnT_p = pool("attnTp", 1)
    gat_p = pool("gatp", 2)
    kst_p = pool("kst", 3)
    w2_p = pool("w2p", 1)

    coef = coef_p.tile([128, NH, LT, NP, 4], BF16, tag="coef")
    cst = coef[:].ap[0][0]
    idxF = coef_p.tile([128, NH, LT, NP], F32, tag="idxF")
    ist = idxF[:].ap[0][0]
    # wrapped-idx staging: free dim = (lH 8, replica 8, j 16) per head
    t16 = coef_p.tile([128, NH, 8, 8, 16], I16, tag="t16")
    idx_wr = coef_p.tile([128, NH * 1024], I16, tag="idx_wr")

    # ---- sampling coordinates, in two l-halves ----
    SH = [128, HLT, 32]
    ost = offaw_n[:].ap[0][0]

    def lhc(t):
        """[128, HLT, 32combo] tile viewed as [part, lt, h, p]."""
        return _tap(t, 0, [[t[:].ap[0][0], 128], [32, HLT], [NP, NH],
                           [1, NP]])

    cp = ctmp_p
    for lh in range(2):
        lt0 = lh * HLT

        def off_view(xy):
            return _tap(offaw_n, lt0 * 96 + xy,
                        [[ost, 128], [96, HLT], [2, 32]])

        def axis_weights(xy, lim):
            g = cp.tile(SH, F32, tag="g")
            gb = _tap(gxb, lt0 * 2 + xy,
                      [[gxb[:].ap[0][0], 128], [2, HLT], [0, 32]])
            nc.vector.tensor_tensor(g[:], off_view(xy), gb, ALU.add)
            # g holds g_true-0.5 (gxb bias -1.0 = grid's -0.5 plus -0.5
            # for round->floor). x0 = round(g) = floor(g_true) via the fp32
            # magic constant (2^23*1.5, representable; at g_true exactly
            # integer the half-even tie gives floor or floor-1, both of
            # which produce identical interpolation).
            x0 = cp.tile(SH, F32, tag="x0")
            nc.vector.tensor_scalar(x0[:], g[:], 12582912.0, 12582912.0,
                                    ALU.add, ALU.subtract)
            fr = cp.tile(SH, F32, tag="t1", name="fr")
            nc.vector.tensor_tensor(fr[:], g[:], x0[:], ALU.subtract)
            wfrac = cp.tile(SH, F32, tag="wf")
            nc.vector.tensor_scalar(wfrac[:], fr[:], 0.5, None, ALU.add)
            wcmp = cp.tile(SH, F32, tag="wcm")
            nc.vector.tensor_scalar(wcmp[:], fr[:], -1.0, 0.5, ALU.mult,
                                    ALU.add)
            bx = cp.tile(SH, F32, tag=f"bx{xy}")
            nc.vector.tensor_scalar(bx[:], x0[:], 0.0, float(lim), ALU.max,
                                    ALU.min)
            d = cp.tile(SH, F32, tag="d")
            nc.vector.tensor_tensor(d[:], bx[:], x0[:], ALU.subtract)
            e0 = cp.tile(SH, F32, tag="e0")
            nc.vector.tensor_scalar(e0[:], d[:], 0.0, None, ALU.is_equal)
            em = cp.tile(SH, F32, tag="em")
            nc.vector.tensor_scalar(em[:], d[:], 1.0, None, ALU.is_equal)
            ep = cp.tile(SH, F32, tag="ep")
            nc.vector.tensor_scalar(ep[:], d[:], -1.0, None, ALU.is_equal)
            t1 = cp.tile(SH, F32, tag="t1")
            s0 = cp.tile(SH, F32, tag=f"s0{xy}")
            nc.vector.tensor_tensor(t1[:], e0[:], wcmp[:], ALU.mult)
            nc.vector.tensor_tensor(s0[:], em[:], wfrac[:], ALU.mult)
            nc.vector.tensor_tensor(s0[:], s0[:], t1[:], ALU.add)
            s1 = cp.tile(SH, F32, tag=f"s1{xy}")
            nc.vector.tensor_tensor(t1[:], e0[:], wfrac[:], ALU.mult)
            nc.vector.tensor_tensor(s1[:], ep[:], wcmp[:], ALU.mult)
            nc.vector.tensor_tensor(s1[:], s1[:], t1[:], ALU.add)
            return s0, s1, bx

        sx0, sx1, bxx = axis_weights(0, W - 2)
        sy0, sy1, bxy = axis_weights(1, H - 2)

        idxf = cp.tile(SH, F32, tag="g", name="idxf")
        nc.vector.tensor_scalar(idxf[:], bxy[:], float(W), 66.0, ALU.mult,
                                ALU.add)
        nc.vector.tensor_tensor(idxf[:], idxf[:], bxx[:], ALU.add)
        idx_dst = _tap(idxF, lt0 * NP,
                       [[ist, 128], [NP, HLT], [LT * NP, NH], [1, NP]])
        nc.vector.tensor_copy(idx_dst, lhc(idxf))

        # softmax over p
        aw4 = _tap(offaw_n, lt0 * 96 + 64,
                   [[ost, 128], [96, HLT], [4, NH], [1, NP]])
        mx = cp.tile([128, HLT, NH], F32, tag="em", name="mx")
        nc.vector.tensor_reduce(mx[:], aw4, AX.X, ALU.max)
        mxb = _tap(mx, 0, [[mx[:].ap[0][0], 128], [NH, HLT], [1, NH],
                           [0, NP]])
        z = cp.tile(SH, F32, tag="x0", name="z")
        zv = _tap(z, 0, [[z[:].ap[0][0], 128], [32, HLT], [4, NH], [1, NP]])
        nc.vector.tensor_tensor(zv, aw4, mxb, ALU.subtract)
        ez = cp.tile(SH, F32, tag="d", name="ez")
        nc.scalar.activation(ez[:], z[:], ACTF.Exp)
        ezv = _tap(ez, 0, [[ez[:].ap[0][0], 128], [32, HLT], [4, NH],
                           [1, NP]])
        ssum = cp.tile([128, HLT, NH], F32, tag="ep", name="ssum")
        nc.vector.tensor_reduce(ssum[:], ezv, AX.X, ALU.add)
        rs = cp.tile([128, HLT, NH], F32, tag="t1", name="rs")
        nc.vector.reciprocal(rs[:], ssum[:])
        rsb = _tap(rs, 0, [[rs[:].ap[0][0], 128], [NH, HLT], [1, NH],
                           [0, NP]])
        Aw = cp.tile(SH, F32, tag="e0", name="Aw")
        Av = _tap(Aw, 0, [[Aw[:].ap[0][0], 128], [32, HLT], [4, NH], [1, NP]])
        nc.vector.tensor_tensor(Av, ezv, rsb, ALU.mult)

        tprod = cp.tile(SH, F32, tag="wf", name="tprod")
        for slot, (sa, sb) in enumerate([(sx0, sy0), (sx1, sy0), (sx0, sy1),
                                         (sx1, sy1)]):
            nc.vector.tensor_tensor(tprod[:], sa[:], sb[:], ALU.mult)
            cdst = AP(coef[:].tensor, coef[:].offset + lt0 * NP * 4 + slot,
                      [[cst, 128], [NP * 4, HLT], [LT * NP * 4, NH], [4, NP]])
            nc.vector.tensor_tensor(cdst, lhc(tprod), lhc(Aw), ALU.mult)

    # ---- idx shuffle: PE transpose -> replicated i16 -> DRAM -> xbar read
    # For head h the gather consumes stream i = lt*512 + pp*128 + l_lo in a
    # 16-wrap [j=i%16 partition, c=i//16 col]. Transposing idxF puts (lt,pp)
    # on partitions; the l_lo free dim then splits as (lH, j) and a stride-0
    # read replicates the 16-value groups for the 8 Q7 core pairs. One
    # contiguous 2KB-run DMA stages [8192, 128] i16 in DRAM; one hardware
    # xbar transpose lands the wrapped layout in SBUF.
    tst = t16[:].ap[0][0]
    for h in range(NH):
        pst = ps_tr.tile([128, 128], F32, tag="pst", name="pidx")
        nc.tensor.transpose(
            pst[:], _tap(idxF, h * LT * NP, [[ist, 128], [NP, LT], [1, NP]]),
            ident[:])
        nc.vector.tensor_copy(
            _tap(t16, h * 1024, [[tst, 128], [128, 8], [16, 8], [1, 16]]),
            _tap(pst, 0, [[pst[:].ap[0][0], 128], [16, 8], [0, 8], [1, 16]]))
    nc.sync.dma_start(
        _tap(idx_dram, 0, [[1024, 128], [128 * 1024, NH], [1, 1024]]),
        t16[:])
    nc.sync.dma_start_transpose(
        idx_wr[:], _tap(idx_dram, 0, [[128, NH * 1024], [1, 128]]))

    if PH < 4:
        w2_p.release(); kst_p.release(); gat_p.release()
        attnT_p.release(); apt_p.release(); pre_p.release()
        coef_p.release(); ctmp_p.release(); dram_p.release()
        ps_tr.release(); ps_misc.release(); ps_conv.release()
        st2_p.release(); base_p.release()
        return

    # ---- gather + weighted reduce + transpose ----
    attn_preT = []
    for hg in range(2):
        pre = pre_p.tile([128, LT, 4, DH], F32, tag="pre")
        prest = pre[:].ap[0][0]
        for hi in range(4):
            h = hg * 4 + hi
            for q in range(4):      # quarter-head chunks: 8 l-tiles each
                g = gat_p.tile([128, 32, 128], BF16, tag="gat")
                idxs = idx_wr[:, h * 1024 + q * 256:h * 1024 + (q + 1) * 256]
                nc.gpsimd.dma_gather(
                    g[:],
                    _ap(v4[h], 0, [[128, 4097], [1, 128]]),
                    idxs, 4096, 4096, 128, single_packet=False,
                    queue_num=q)
                gst = g[:].ap[0][0]
                gv = _tap(g, 0, [[gst, 128], [512, 8], [128, NP], [32, 4],
                                 [1, DH]])
                cch = AP(coef[:].tensor,
                         coef[:].offset + h * LT * NP * 4 + q * 8 * NP * 4,
                         [[cst, 128], [16, 8], [4, NP], [1, 4], [0, DH]])
                nc.vector.tensor_tensor(gv, gv, cch, ALU.mult)
                gr = _tap(g, 0, [[gst, 128], [512, 8], [1, DH], [32, 16]])
                nc.vector.tensor_reduce(
                    _tap(pre, (q * 8) * 4 * DH + hi * DH,
                         [[prest, 128], [4 * DH, 8], [1, DH]]),
                    gr, AX.X, ALU.add)
        # transpose [128 l_lo, 128 (4 heads x 32 dh)] per l-tile -> bf16
        apt = apt_p.tile([128, L], BF16, tag="apT", name="apt")
        attn_preT.append(apt)
        for lt in range(LT):
            pst = ps_tr.tile([128, 128], F32, tag="pst")
            nc.tensor.transpose(pst[:], pre[:, lt, :, :], ident[:])
            nc.vector.tensor_copy(apt[:, lt * 128:(lt + 1) * 128], pst[:])

    if PH < 5:
        w2_p.release(); kst_p.release(); gat_p.release()
        attnT_p.release(); apt_p.release(); pre_p.release()
        coef_p.release(); ctmp_p.release(); dram_p.release()
        ps_tr.release(); ps_misc.release(); ps_conv.release()
        st2_p.release(); base_p.release()
        return

    # ---- attn out-projection (bf16, +out_b) ----
    attnT_bf = attnT_p.tile([128, 2, L], BF16, tag="attnT")
    for mg in range(2):
        for n in range(NT):
            ps = ps_misc.tile([128, 512], F32, tag="psv")
            for k in range(2):
                nc.tensor.matmul(
                    ps[:],
                    outw[:, k, mg * 128:(mg + 1) * 128],
                    attn_preT[k][:, n * 512:(n + 1) * 512],
                    start=(k == 0), stop=(k == 1))
            nc.scalar.activation(attnT_bf[:, mg, n * 512:(n + 1) * 512],
                                 ps[:], ACTF.Identity,
                                 bias=wbias[:, mg:mg + 1])

    # ---- cv2 ----
    w2a = w2_p.tile([128, 5, C2], BF16, tag="w2a")
    nc.sync.dma_start(w2a[:], _ap(w2d, 0, [[C2, 128], [128 * C2, 5], [1, C2]]))
    w2b = w2_p.tile([128, 5, C2], BF16, tag="w2b")
    nc.sync.dma_start(w2b[:],
                      _ap(w2d, 5 * 128 * C2, [[C2, 128], [128 * C2, 5],
                                              [1, C2]]))

    for n in range(NT):
        ktiles = []
        for kk in range(8):
            t = kst_p.tile([128, 512], BF16, tag="kstream")
            nc.sync.dma_start(
                t[:], _tap(bf_dram, kk * 128 * L + n * 512,
                           [[L, 128], [1, 512]]))
            ktiles.append(t)
        for m in range(4):
            ps = ps_conv.tile([128, 512], F32, tag="conv_ps")
            for k in range(10):
                rhs = (ktiles[k][:] if k < 8
                       else attnT_bf[:, k - 8, n * 512:(n + 1) * 512])
                wt = w2a if k < 5 else w2b
                nc.tensor.matmul(ps[:], wt[:, k % 5, m * 128:(m + 1) * 128],
                                 rhs, start=(k == 0), stop=(k == 9))
            o = st2_p.tile([128, 512], F32, tag="st2", name="o")
            nc.scalar.activation(o[:], ps[:], ACT_MAIN)
            nc.sync.dma_start(
                _ap(outd, m * 128 * L + n * 512, [[L, 128], [1, 512]]), o[:])

    w2_p.release()
    kst_p.release()
    gat_p.release()
    attnT_p.release()
    apt_p.release()
    pre_p.release()
    coef_p.release()
    ctmp_p.release()
    dram_p.release()
    ps_tr.release()
    ps_misc.release()
    ps_conv.release()
    st2_p.release()
    base_p.release()


def host_prep(inputs):
    import ml_dtypes
    x = np.asarray(inputs["x"], np.float32).reshape(B, C1, L)
    rb = np.asarray(inputs["refer_bbox"], np.float32).reshape(B, L, 2)
    w1t = np.ascontiguousarray(
        np.asarray(inputs["cv1_w"], np.float32)[:, :, 0, 0].T)
    wc = np.ascontiguousarray(np.stack([
        np.asarray(inputs[k], np.float32).transpose(2, 3, 1, 0).reshape(
            9, C, C)
        for k in ["m0_cv1_w", "m0_cv2_w", "m1_cv1_w", "m1_cv2_w"]]))
    w2t = np.ascontiguousarray(
        np.asarray(inputs["cv2_w"], np.float32)[:, :, 0, 0].T).astype(
            ml_dtypes.bfloat16)
    out_w = np.ascontiguousarray(
        np.asarray(inputs["out_w"], np.float32)).astype(ml_dtypes.bfloat16)
    shared = {
        "w1t": w1t, "wc": wc, "w2t": w2t, "out_w": out_w,
        "vproj_w": np.ascontiguousarray(
            np.asarray(inputs["vproj_w"], np.float32)),
        "offaw_w": np.ascontiguousarray(np.concatenate(
            [np.asarray(inputs["off_w"], np.float32),
             np.asarray(inputs["aw_w"], np.float32)], axis=1)),
        "vproj_b": np.asarray(inputs["vproj_b"], np.float32).reshape(1, D),
        "offaw_b": np.ascontiguousarray(np.concatenate(
            [np.asarray(inputs["off_b"], np.float32),
             np.asarray(inputs["aw_b"], np.float32)]).reshape(1, 96)),
        "out_b": np.asarray(inputs["out_b"], np.float32).reshape(D, 1),
    }
    in_maps = []
    for c in range(B):
        m = dict(shared)
        m["x"] = np.ascontiguousarray(x[c])
        m["refer"] = np.ascontiguousarray(rb[c])
        in_maps.append(m)
    return in_maps


def kernel(**inputs):
    nc = build(B)
    in_maps = host_prep(inputs)
    res = run_bass_kernel_spmd(nc, in_maps, core_ids=list(range(B)))
    out = np.stack([res.results[c]["out"].reshape(C2, H, W) for c in range(B)])
    return out.astype(np.float32)


if __name__ == "__main__":
    build()
    print("build ok")



# revision 15
# speedup vs baseline: 5.4747x; 1.6647x over previous
"""Trainium2 Bass kernel for C2f-with-DeformableAttention block.

Sharding: data-parallel over batch (8 images -> 8 NeuronCores), weights
replicated, no collectives. Each core runs the full block for one image:
  cv1 (1x1) -> split a/b -> 2x Bottleneck(3x3+3x3) -> msdeform attn
  -> concat(a,b,b1,b2,attn) -> cv2 (1x1), SiLU after every conv.

Per-core layouts:
  feature maps: channel-major [C partitions, H*W free]; 3x3-conv inputs are
  zero-padded [C, 66*66] so the 9 taps are contiguous shifted reads feeding
  PSUM-accumulated matmuls.
  deformable sampling: a per-head V4 table in DRAM packs the 4 bilinear
  corner pixels per base index into 512B rows, fetched by SWDGE dma_gather;
  bilinear + softmax attention weights fold into 16 coefficients per sample
  applied with one multiply + one strided reduce on VectorE. Border clipping
  is folded into the coefficients (base index clamped to [0,62]^2, weights
  remapped/zeroed), matching grid_sample(align_corners=False) + masking.
Matmuls run float32r (full-rate fp32 PE mode, fp32 PSUM accumulation); the
concat/cv2 and attention-output paths are bf16. SBUF pressure is managed
with two sequential pool scopes (convs, attention) plus tag-based slot
rotation for the large maps.
"""

import os
import sys

sys.path.insert(0, "/opt/trn_rl_repo")

import numpy as np

import concourse.bass as bass
import concourse.tile as tile
from concourse import bacc, mybir
from concourse.bass import AP
from concourse.bass_utils import run_bass_kernel_spmd
from concourse.masks import make_identity

F32 = mybir.dt.float32
F32R = mybir.dt.float32r
BF16 = mybir.dt.bfloat16
I16 = mybir.dt.int16
ALU = mybir.AluOpType
ACTF = mybir.ActivationFunctionType
AX = mybir.AxisListType

B, C1, C2 = 8, 512, 512
C = 256
D = 256
NH, NP = 8, 4
H = W = 64
L = H * W            # 4096
DH = D // NH         # 32
PW = W + 2           # 66
DOFF = 1             # leading pad element so tap offset -1 stays in-tile
PADLEN = PW * 66 + 16   # per-channel padded map length (+DOFF+tail slack)
OUTREG = 64 * PW     # 4224: contiguous output region = rows 1..64 (all cols)
V4ROWS = 4168        # 66 front pad + 4096 rows + tail
LT = L // 128        # 32
HLT = LT // 2        # 16 (coords run in two l-halves)
NT = L // 512        # 8

SIM_ACT = os.environ.get("BASS_KERNEL_SIM_ACT", "") == "sigmoid"
PH = int(os.environ.get("BASS_KERNEL_PHASES", "9"))
ACT_MAIN = ACTF.Sigmoid if SIM_ACT else ACTF.Silu

_cache = {}


def _ap(t, offset, dims):
    """AP into a DRAM tensor handle at element offset."""
    return AP(t.ap().tensor, offset, dims)


def _tap(tile_, offset, dims):
    """AP into an SBUF/DRAM tile at element offset from tile base."""
    a = tile_[:]
    return AP(a.tensor, a.offset + offset, dims)


def build(n_cores=8):
    key = ("nc", SIM_ACT, PH)
    if key in _cache:
        return _cache[key]
    nc = bacc.Bacc("TRN2", target_bir_lowering=False, debug=False,
                   num_devices=n_cores, num_swdge_queues=4)

    xd = nc.dram_tensor("x", [C1, L], BF16, kind="ExternalInput")
    rbd = nc.dram_tensor("refer", [L, 2], F32, kind="ExternalInput")
    w1d = nc.dram_tensor("w1t", [C1, C1], BF16, kind="ExternalInput")
    wcd = nc.dram_tensor("wc", [4, 9, C, C], BF16, kind="ExternalInput")
    w2d = nc.dram_tensor("w2t", [5 * C, C2], BF16, kind="ExternalInput")
    vpd = nc.dram_tensor("vproj_w", [D, D], BF16, kind="ExternalInput")
    oad = nc.dram_tensor("offaw_w", [D, 96], BF16, kind="ExternalInput")
    owd = nc.dram_tensor("out_w", [D, D], BF16, kind="ExternalInput")
    vbd = nc.dram_tensor("vproj_b", [1, D], F32R, kind="ExternalInput")
    obd = nc.dram_tensor("offaw_b", [1, 96], F32R, kind="ExternalInput")
    wbd = nc.dram_tensor("out_b", [D, 1], F32, kind="ExternalInput")
    outd = nc.dram_tensor("out", [C2, L], F32, kind="ExternalOutput")

    with tile.TileContext(nc) as tc:
        _build_tile(nc, tc, xd, rbd, w1d, wcd, w2d, vpd, oad, owd, vbd, obd,
                    wbd, outd)
    nc.compile()
    _cache[key] = nc
    return nc


def _build_tile(nc, tc, xd, rbd, w1d, wcd, w2d, vpd, oad, owd, vbd, obd, wbd,
                outd):
    def pool(name, bufs, space="SBUF"):
        return tc.alloc_tile_pool(name=name, bufs=bufs, space=space)

    # ---- base pools: live for the whole program ----
    base_p = pool("base", 1)
    st2_p = pool("st2", 3)          # [128,512] staging (spills + outputs)
    ps_conv = pool("ps_conv", 4, space="PSUM")
    ps_misc = pool("ps_misc", 2, space="PSUM")
    ps_tr = pool("ps_tr", 2, space="PSUM")
    dram_p = pool("scratch", 1, space="DRAM")

    ident = base_p.tile([128, 128], F32)
    make_identity(nc, ident[:])
    ones1 = base_p.tile([1, 128], F32R)
    nc.vector.memset(ones1[:].bitcast(F32), 1.0)
    vb1 = base_p.tile([1, D], F32R)
    nc.sync.dma_start(vb1[:], vbd.ap())
    vbias = base_p.tile([128, D], F32)
    psb = ps_misc.tile([128, 512], F32, tag="psv", name="psb")
    nc.tensor.matmul(psb[:, :D], ones1[:], vb1[:], start=True, stop=True)
    nc.vector.tensor_copy(vbias[:], psb[:, :D])
    ob1 = base_p.tile([1, 96], F32R)
    nc.sync.dma_start(ob1[:], obd.ap())
    obias = base_p.tile([128, 96], F32)
    psb2 = ps_misc.tile([128, 512], F32, tag="psv", name="psb2")
    nc.tensor.matmul(psb2[:, :96], ones1[:], ob1[:], start=True, stop=True)
    nc.vector.tensor_copy(obias[:], psb2[:, :96])
    wbias = base_p.tile([128, 2], F32)
    nc.sync.dma_start(wbias[:], _ap(wbd, 0, [[1, 128], [128, 2]]))
    vproj = base_p.tile([128, 2, D], BF16)
    nc.sync.dma_start(vproj[:], _ap(vpd, 0, [[D, 128], [128 * D, 2], [1, D]]))
    offaw = base_p.tile([128, 2, 96], BF16)
    nc.sync.dma_start(offaw[:],
                      _ap(oad, 0, [[96, 128], [128 * 96, 2], [1, 96]]))
    outw = base_p.tile([128, 2, D], BF16)
    nc.sync.dma_start(outw[:], _ap(owd, 0, [[D, 128], [128 * D, 2], [1, D]]))
    offaw_n = base_p.tile([128, LT, 96], F32)
    rb = base_p.tile([128, LT, 2], F32)
    nc.sync.dma_start(rb[:], _ap(rbd, 0, [[2, 128], [256, LT], [1, 2]]))
    gxb = base_p.tile([128, LT, 2], F32)
    nc.scalar.activation(gxb[:], rb[:], ACTF.Copy, bias=-1.0, scale=64.0)

    bf_dram = dram_p.tile([8, 128, L], BF16)   # a,b,b1,b2 k-tiles for cv2
    v4 = [nc.dram_tensor(f"v4_{h}", [V4ROWS, 128], BF16, kind="Internal")
          for h in range(NH)]
    # idx staging: [8192 rows, 128 cols] i16; read back via xbar transpose
    idx_dram = dram_p.tile([NH * 128 * 1024], I16)

    def spill_chunk(src_ap, slot_k, n):
        """cast a [128,8,64] f32(r) view to bf16 and store to bf_dram."""
        t = st2_p.tile([128, 512], BF16, tag="st2", name="spl")
        dst = _tap(t, 0, [[512, 128], [64, 8], [1, 64]])
        nc.vector.tensor_copy(dst, src_ap)
        nc.sync.dma_start(
            _tap(bf_dram, slot_k * 128 * L + n * 512, [[L, 128], [1, 512]]),
            t[:])

    def spill_map(src_view_fn, slot):
        """spill a 256-ch map (two [128, 64rows, 64] views) to bf_dram."""
        for k in range(2):
            v = src_view_fn(k)
            for n in range(NT):
                sub = AP(v.tensor, v.offset + (n * 8) * v.ap[1][0],
                         [[v.ap[0][0], 128], [v.ap[1][0], 8], [1, 64]])
                spill_chunk(sub, slot * 2 + k, n)

    # ================= scope 1: cv1 + bottlenecks + projections ==========
    s1_p = pool("s1", 1)      # xt
    wc_p = pool("wcp", 2)     # conv weight halves (9KB slots)
    big_p = pool("bigp", 2)   # 35KB slots: pads, b2, value (rotating)

    xt = s1_p.tile([128, 4, L], BF16, tag="xt")
    for n in range(NT):
        nc.sync.dma_start(
            _tap(xt, n * 512, [[4 * L, 128], [L, 4], [1, 512]]),
            _ap(xd, n * 512, [[L, 128], [128 * L, 4], [1, 512]]))
    w1 = wc_p.tile([128, 4, C1], BF16, tag="wc", name="w1")
    nc.sync.dma_start(w1[:], _ap(w1d, 0, [[C1, 128], [128 * C1, 4], [1, C1]]))

    b_pad = big_p.tile([128, 2, PADLEN], BF16, tag="big", name="b_pad")
    nc.vector.memset(b_pad[:], 0.0)

    for m in range(4):
        for n in range(NT):
            ps = ps_conv.tile([128, 512], F32, tag="conv_ps")
            for k in range(4):
                nc.tensor.matmul(
                    ps[:],
                    w1[:, k, m * 128:(m + 1) * 128],
                    xt[:, k, n * 512:(n + 1) * 512],
                    start=(k == 0), stop=(k == 3))
            if m < 2:
                # 'a' goes straight to DRAM as bf16 (k-tile slot m)
                t = st2_p.tile([128, 512], BF16, tag="st2", name="a_st")
                nc.scalar.activation(t[:], ps[:], ACT_MAIN)
                nc.sync.dma_start(
                    _tap(bf_dram, m * 128 * L + n * 512, [[L, 128], [1, 512]]),
                    t[:])
            else:
                # scatter 512 pixels = 8 rows of 64 into the padded layout
                row0 = n * 8
                dst = _tap(b_pad,
                           (m - 2) * PADLEN + DOFF + (row0 + 1) * PW + 1,
                           [[b_pad[:].ap[0][0], 128], [PW, 8], [1, 64]])
                src = _tap(ps, 0, [[ps[:].ap[0][0], 128], [64, 8], [1, 64]])
                nc.scalar.activation(dst, src, ACT_MAIN)

    # ---- bottleneck convs ----
    wc_tiles = {}

    def load_wc(ci, m):
        t = wc_p.tile([128, 9, 2, 128], BF16, tag="wc", name="wch")
        nc.sync.dma_start(
            t[:], _ap(wcd, ci * 9 * C * C + m * 128,
                      [[C, 128], [C * C, 9], [128 * C, 2], [1, 128]]))
        wc_tiles[(ci, m)] = t

    wcseq = [(ci, m) for ci in range(4) for m in range(2)]
    load_wc(0, 0)

    def conv3x3(src, ci, dst_fn, chunks=None):
        """src: padded [128,2,PADLEN] tile. dst_fn(m, pos, nsz, psum).
        pos/nsz index the 4224-long out region (padded idx DOFF+66+o)."""
        sst = src[:].ap[0][0]
        if chunks is None:
            chunks = [(i * 512, min(512, OUTREG - i * 512)) for i in range(9)]
        for m in range(2):
            wt = wc_tiles[(ci, m)]
            nxt = wcseq.index((ci, m)) + 1
            if nxt < len(wcseq) and wcseq[nxt] not in wc_tiles:
                load_wc(*wcseq[nxt])
            for pos, nsz in chunks:
                ps = ps_conv.tile([128, 512], F32, tag="conv_ps")
                i = 0
                for tap in range(9):
                    ty, tx = tap // 3, tap % 3
                    off = DOFF + pos + ty * PW + tx - 1
                    for k in range(2):
                        nc.tensor.matmul(
                            ps[:, :nsz],
                            wt[:, tap, k, :],
                            _tap(src, k * PADLEN + off,
                                 [[sst, 128], [1, nsz]]),
                            start=(i == 0), stop=(i == 17))
                        i += 1
                dst_fn(m, pos, nsz, ps)

    def pad_writer(dst):
        def f(m, pos, nsz, ps):
            nc.scalar.activation(
                _tap(dst, m * PADLEN + DOFF + PW + pos,
                     [[dst[:].ap[0][0], 128], [1, nsz]]),
                ps[:, :nsz], ACT_MAIN)
        return f

    def zero_padcols(t):
        nc.vector.memset(
            _tap(t, DOFF + PW, [[t[:].ap[0][0], 128], [PADLEN, 2], [PW, 64],
                                [65, 2]]), 0.0)

    def padded_view(t, k):
        return _tap(t, k * PADLEN + DOFF + PW + 1,
                    [[t[:].ap[0][0], 128], [PW, 64], [1, 64]])

    mid = big_p.tile([128, 2, PADLEN], BF16, tag="big", name="mid")
    nc.vector.memset(mid[:], 0.0)
    conv3x3(b_pad, 0, pad_writer(mid))
    zero_padcols(mid)
    spill_map(lambda k: padded_view(b_pad, k), 1)

    b1_pad = big_p.tile([128, 2, PADLEN], BF16, tag="big", name="b1_pad")
    nc.vector.memset(b1_pad[:], 0.0)
    conv3x3(mid, 1, pad_writer(b1_pad))
    zero_padcols(b1_pad)

    mid2 = big_p.tile([128, 2, PADLEN], BF16, tag="big", name="mid2")
    nc.vector.memset(mid2[:], 0.0)
    conv3x3(b1_pad, 2, pad_writer(mid2))
    zero_padcols(mid2)
    spill_map(lambda k: padded_view(b1_pad, k), 2)

    b2 = big_p.tile([128, 2, PADLEN], BF16, tag="big", name="b2")

    def b2_writer(m, pos, nsz, ps):
        row0, nrow = pos // PW, nsz // PW
        dst = _tap(b2, m * PADLEN + row0 * 64,
                   [[b2[:].ap[0][0], 128], [64, nrow], [1, 64]])
        src = _tap(ps, 1, [[ps[:].ap[0][0], 128], [PW, nrow], [1, 64]])
        nc.scalar.activation(dst, src, ACT_MAIN)

    rowchunks = [(rc * 4 * PW, 4 * PW) for rc in range(16)]  # 264 each
    conv3x3(mid2, 3, b2_writer, chunks=rowchunks)

    if PH < 2:
        big_p.release(); wc_p.release(); s1_p.release()
        dram_p.release(); ps_tr.release(); ps_misc.release()
        ps_conv.release(); st2_p.release(); base_p.release()
        return
    # ---- projections ----
    value = big_p.tile([128, 2, PADLEN], BF16, tag="big", name="value")
    vst = value[:].ap[0][0]

    for lt in range(LT):
        psv = ps_misc.tile([128, 512], F32, tag="psv")
        for k in range(2):
            nc.tensor.matmul(psv[:, :D],
                             _tap(b2, k * PADLEN + lt * 128,
                                  [[b2[:].ap[0][0], 128], [1, 128]]),
                             vproj[:, k, :],
                             start=(k == 0), stop=(k == 1))
        nc.vector.tensor_tensor(_tap(value, lt * D, [[vst, 128], [1, D]]),
                                psv[:, :D], vbias[:], ALU.add)
        pso = ps_misc.tile([128, 512], F32, tag="psv")
        for k in range(2):
            nc.tensor.matmul(pso[:, :96],
                             _tap(b2, k * PADLEN + lt * 128,
                                  [[b2[:].ap[0][0], 128], [1, 128]]),
                             offaw[:, k, :],
                             start=(k == 0), stop=(k == 1))
        nc.vector.tensor_tensor(offaw_n[:, lt, :], pso[:, :96], obias[:],
                                ALU.add)
    spill_map(lambda k: _tap(b2, k * PADLEN,
                             [[b2[:].ap[0][0], 128], [64, 64], [1, 64]]), 3)

    # V4: per head h, row (66+i) = [V[i], V[i+1], V[i+64], V[i+65]]
    for hh in range(NH):
        for si, dlt in enumerate([0, 1, 64, 65]):
            src = _tap(value, hh * DH, [[vst, 128], [D, LT], [1, DH]])
            dst = _ap(v4[hh], (66 - dlt) * 128 + si * DH,
                      [[128, 128], [128 * 128, LT], [1, DH]])
            nc.sync.dma_start(dst, src)

    big_p.release()
    wc_p.release()
    s1_p.release()

    if PH < 3:
        dram_p.release(); ps_tr.release(); ps_misc.release()
        ps_conv.release(); st2_p.release(); base_p.release()
        return

    # ================= scope 2: coords + gather + attn + cv2 =============
    ctmp_p = pool("ctmp", 1)
    coef_p = pool("coefp", 1)
    pre_p = pool("prep", 1)
    apt_p = pool("aptp", 2)
    attnT_p = pool("attnTp", 1)
    gat_p = pool("gatp", 4)
    xp_p = pool("xp", 2)
    kst_p = pool("kst", 3)
    w2_p = pool("w2p", 1)

    coef = coef_p.tile([128, NH, LT, NP, 4], BF16, tag="coef")
    cst = coef[:].ap[0][0]
    idxF = coef_p.tile([128, NH, LT, NP], F32, tag="idxF")
    ist = idxF[:].ap[0][0]
    # wrapped-idx staging: free dim = (lH 8, replica 8, j 16) per head
    t16 = coef_p.tile([128, NH, 8, 8, 16], I16, tag="t16")
    idx_wr = coef_p.tile([128, NH * 1024], I16, tag="idx_wr")

    # ---- sampling coordinates, in two l-halves ----
    SH = [128, HLT, 32]
    ost = offaw_n[:].ap[0][0]

    def lhc(t):
        """[128, HLT, 32combo] tile viewed as [part, lt, h, p]."""
        return _tap(t, 0, [[t[:].ap[0][0], 128], [32, HLT], [NP, NH],
                           [1, NP]])

    cp = ctmp_p
    for lh in range(2):
        lt0 = lh * HLT

        def off_view(xy):
            return _tap(offaw_n, lt0 * 96 + xy,
                        [[ost, 128], [96, HLT], [2, 32]])

        def axis_weights(xy, lim):
            g = cp.tile(SH, F32, tag="g")
            gb = _tap(gxb, lt0 * 2 + xy,
                      [[gxb[:].ap[0][0], 128], [2, HLT], [0, 32]])
            nc.vector.tensor_tensor(g[:], off_view(xy), gb, ALU.add)
            # g holds g_true-0.5 (gxb bias -1.0 = grid's -0.5 plus -0.5
            # for round->floor). x0 = round(g) = floor(g_true) via the fp32
            # magic constant (2^23*1.5, representable; at g_true exactly
            # integer the half-even tie gives floor or floor-1, both of
            # which produce identical interpolation).
            x0 = cp.tile(SH, F32, tag="x0")
            nc.vector.tensor_scalar(x0[:], g[:], 12582912.0, 12582912.0,
                                    ALU.add, ALU.subtract)
            fr = cp.tile(SH, F32, tag="t1", name="fr")
            nc.vector.tensor_tensor(fr[:], g[:], x0[:], ALU.subtract)
            wfrac = cp.tile(SH, F32, tag="wf")
            nc.vector.tensor_scalar(wfrac[:], fr[:], 0.5, None, ALU.add)
            wcmp = cp.tile(SH, F32, tag="wcm")
            nc.vector.tensor_scalar(wcmp[:], fr[:], -1.0, 0.5, ALU.mult,
                                    ALU.add)
            bx = cp.tile(SH, F32, tag=f"bx{xy}")
            nc.vector.tensor_scalar(bx[:], x0[:], 0.0, float(lim), ALU.max,
                                    ALU.min)
            d = cp.tile(SH, F32, tag="d")
            nc.vector.tensor_tensor(d[:], bx[:], x0[:], ALU.subtract)
            e0 = cp.tile(SH, F32, tag="e0")
            nc.vector.tensor_scalar(e0[:], d[:], 0.0, None, ALU.is_equal)
            em = cp.tile(SH, F32, tag="em")
            nc.vector.tensor_scalar(em[:], d[:], 1.0, None, ALU.is_equal)
            ep = cp.tile(SH, F32, tag="ep")
            nc.vector.tensor_scalar(ep[:], d[:], -1.0, None, ALU.is_equal)
            t1 = cp.tile(SH, F32, tag="t1")
            s0 = cp.tile(SH, F32, tag=f"s0{xy}")
            nc.vector.tensor_tensor(t1[:], e0[:], wcmp[:], ALU.mult)
            nc.vector.tensor_tensor(s0[:], em[:], wfrac[:], ALU.mult)
            nc.vector.tensor_tensor(s0[:], s0[:], t1[:], ALU.add)
            s1 = cp.tile(SH, F32, tag=f"s1{xy}")
            nc.vector.tensor_tensor(t1[:], e0[:], wfrac[:], ALU.mult)
            nc.vector.tensor_tensor(s1[:], ep[:], wcmp[:], ALU.mult)
            nc.vector.tensor_tensor(s1[:], s1[:], t1[:], ALU.add)
            return s0, s1, bx

        sx0, sx1, bxx = axis_weights(0, W - 2)
        sy0, sy1, bxy = axis_weights(1, H - 2)

        idxf = cp.tile(SH, F32, tag="g", name="idxf")
        nc.vector.tensor_scalar(idxf[:], bxy[:], float(W), 66.0, ALU.mult,
                                ALU.add)
        nc.vector.tensor_tensor(idxf[:], idxf[:], bxx[:], ALU.add)
        idx_dst = _tap(idxF, lt0 * NP,
                       [[ist, 128], [NP, HLT], [LT * NP, NH], [1, NP]])
        nc.vector.tensor_copy(idx_dst, lhc(idxf))

        # softmax over p
        aw4 = _tap(offaw_n, lt0 * 96 + 64,
                   [[ost, 128], [96, HLT], [4, NH], [1, NP]])
        mx = cp.tile([128, HLT, NH], F32, tag="em", name="mx")
        nc.vector.tensor_reduce(mx[:], aw4, AX.X, ALU.max)
        mxb = _tap(mx, 0, [[mx[:].ap[0][0], 128], [NH, HLT], [1, NH],
                           [0, NP]])
        z = cp.tile(SH, F32, tag="x0", name="z")
        zv = _tap(z, 0, [[z[:].ap[0][0], 128], [32, HLT], [4, NH], [1, NP]])
        nc.vector.tensor_tensor(zv, aw4, mxb, ALU.subtract)
        ez = cp.tile(SH, F32, tag="d", name="ez")
        nc.scalar.activation(ez[:], z[:], ACTF.Exp)
        ezv = _tap(ez, 0, [[ez[:].ap[0][0], 128], [32, HLT], [4, NH],
                           [1, NP]])
        ssum = cp.tile([128, HLT, NH], F32, tag="ep", name="ssum")
        nc.vector.tensor_reduce(ssum[:], ezv, AX.X, ALU.add)
        rs = cp.tile([128, HLT, NH], F32, tag="t1", name="rs")
        nc.vector.reciprocal(rs[:], ssum[:])
        rsb = _tap(rs, 0, [[rs[:].ap[0][0], 128], [NH, HLT], [1, NH],
                           [0, NP]])
        Aw = cp.tile(SH, F32, tag="e0", name="Aw")
        Av = _tap(Aw, 0, [[Aw[:].ap[0][0], 128], [32, HLT], [4, NH], [1, NP]])
        nc.vector.tensor_tensor(Av, ezv, rsb, ALU.mult)

        tprod = cp.tile(SH, F32, tag="wf", name="tprod")
        for slot, (sa, sb) in enumerate([(sx0, sy0), (sx1, sy0), (sx0, sy1),
                                         (sx1, sy1)]):
            nc.vector.tensor_tensor(tprod[:], sa[:], sb[:], ALU.mult)
            cdst = AP(coef[:].tensor, coef[:].offset + lt0 * NP * 4 + slot,
                      [[cst, 128], [NP * 4, HLT], [LT * NP * 4, NH], [4, NP]])
            nc.vector.tensor_tensor(cdst, lhc(tprod), lhc(Aw), ALU.mult)

    # ---- idx shuffle: PE transpose -> replicated i16 -> DRAM -> xbar read
    # For head h the gather consumes stream i = lt*512 + pp*128 + l_lo in a
    # 16-wrap [j=i%16 partition, c=i//16 col]. Transposing idxF puts (lt,pp)
    # on partitions; the l_lo free dim then splits as (lH, j) and a stride-0
    # read replicates the 16-value groups for the 8 Q7 core pairs. One
    # contiguous 2KB-run DMA stages [8192, 128] i16 in DRAM; one hardware
    # xbar transpose lands the wrapped layout in SBUF.
    tst = t16[:].ap[0][0]
    for h in range(NH):
        pst = ps_tr.tile([128, 128], F32, tag="pst", name="pidx")
        nc.tensor.transpose(
            pst[:], _tap(idxF, h * LT * NP, [[ist, 128], [NP, LT], [1, NP]]),
            ident[:])
        nc.vector.tensor_copy(
            _tap(t16, h * 1024, [[tst, 128], [128, 8], [16, 8], [1, 16]]),
            _tap(pst, 0, [[pst[:].ap[0][0], 128], [16, 8], [0, 8], [1, 16]]))
    nc.sync.dma_start(
        _tap(idx_dram, 0, [[1024, 128], [128 * 1024, NH], [1, 1024]]),
        t16[:])
    nc.sync.dma_start_transpose(
        idx_wr[:], _tap(idx_dram, 0, [[128, NH * 1024], [1, 128]]))

    if PH < 4:
        w2_p.release(); kst_p.release(); xp_p.release()
        gat_p.release()
        attnT_p.release(); apt_p.release(); pre_p.release()
        coef_p.release(); ctmp_p.release(); dram_p.release()
        ps_tr.release(); ps_misc.release(); ps_conv.release()
        st2_p.release(); base_p.release()
        return

    # ---- gather + weighted reduce + transpose ----
    attn_preT = []
    for hg in range(2):
        pre = pre_p.tile([128, LT, 4, DH], F32, tag="pre")
        prest = pre[:].ap[0][0]
        for hi in range(4):
            h = hg * 4 + hi
            for q in range(4):      # quarter-head chunks: 8 l-tiles each
                g = gat_p.tile([128, 32, 128], BF16, tag="gat")
                idxs = idx_wr[:, h * 1024 + q * 256:h * 1024 + (q + 1) * 256]
                nc.gpsimd.dma_gather(
                    g[:],
                    _ap(v4[h], 0, [[128, 4097], [1, 128]]),
                    idxs, 4096, 4096, 128, single_packet=False,
                    queue_num=q)
                gst = g[:].ap[0][0]
                gv = _tap(g, 0, [[gst, 128], [512, 8], [128, NP], [32, 4],
                                 [1, DH]])
                cch = AP(coef[:].tensor,
                         coef[:].offset + h * LT * NP * 4 + q * 8 * NP * 4,
                         [[cst, 128], [16, 8], [4, NP], [1, 4], [0, DH]])
                nc.vector.tensor_tensor(gv, gv, cch, ALU.mult)
                # pairwise-add tree over the 16 (point, corner) slots:
                # bf16 tensor_tensor runs 2x/cycle; tensor_reduce is 1x-only.
                x1 = xp_p.tile([128, 8, 8, DH], BF16, tag="x1")
                s1 = x1[:].ap[0][0]
                nc.vector.tensor_tensor(
                    x1[:],
                    _tap(g, 0, [[gst, 128], [512, 8], [64, 8], [1, DH]]),
                    _tap(g, DH, [[gst, 128], [512, 8], [64, 8], [1, DH]]),
                    ALU.add)
                x2 = xp_p.tile([128, 8, 4, DH], BF16, tag="x2")
                s2 = x2[:].ap[0][0]
                nc.vector.tensor_tensor(
                    x2[:],
                    _tap(x1, 0, [[s1, 128], [256, 8], [64, 4], [1, DH]]),
                    _tap(x1, DH, [[s1, 128], [256, 8], [64, 4], [1, DH]]),
                    ALU.add)
                x3 = xp_p.tile([128, 8, 2, DH], BF16, tag="x3")
                s3 = x3[:].ap[0][0]
                nc.vector.tensor_tensor(
                    x3[:],
                    _tap(x2, 0, [[s2, 128], [128, 8], [64, 2], [1, DH]]),
                    _tap(x2, DH, [[s2, 128], [128, 8], [64, 2], [1, DH]]),
                    ALU.add)
                nc.vector.tensor_tensor(
                    _tap(pre, (q * 8) * 4 * DH + hi * DH,
                         [[prest, 128], [4 * DH, 8], [1, DH]]),
                    _tap(x3, 0, [[s3, 128], [64, 8], [1, DH]]),
                    _tap(x3, DH, [[s3, 128], [64, 8], [1, DH]]),
                    ALU.add)
        # transpose [128 l_lo, 128 (4 heads x 32 dh)] per l-tile -> bf16
        apt = apt_p.tile([128, L], BF16, tag="apT", name="apt")
        attn_preT.append(apt)
        for lt in range(LT):
            pst = ps_tr.tile([128, 128], F32, tag="pst")
            nc.tensor.transpose(pst[:], pre[:, lt, :, :], ident[:])
            nc.vector.tensor_copy(apt[:, lt * 128:(lt + 1) * 128], pst[:])

    if PH < 5:
        w2_p.release(); kst_p.release(); xp_p.release()
        gat_p.release()
        attnT_p.release(); apt_p.release(); pre_p.release()
        coef_p.release(); ctmp_p.release(); dram_p.release()
        ps_tr.release(); ps_misc.release(); ps_conv.release()
        st2_p.release(); base_p.release()
        return

    # ---- attn out-projection (bf16, +out_b) ----
    attnT_bf = attnT_p.tile([128, 2, L], BF16, tag="attnT")
    for mg in range(2):
        for n in range(NT):
            ps = ps_misc.tile([128, 512], F32, tag="psv")
            for k in range(2):
                nc.tensor.matmul(
                    ps[:],
                    outw[:, k, mg * 128:(mg + 1) * 128],
                    attn_preT[k][:, n * 512:(n + 1) * 512],
                    start=(k == 0), stop=(k == 1))
            nc.scalar.activation(attnT_bf[:, mg, n * 512:(n + 1) * 512],
                                 ps[:], ACTF.Identity,
                                 bias=wbias[:, mg:mg + 1])

    # ---- cv2 ----
    w2a = w2_p.tile([128, 5, C2], BF16, tag="w2a")
    nc.sync.dma_start(w2a[:], _ap(w2d, 0, [[C2, 128], [128 * C2, 5], [1, C2]]))
    w2b = w2_p.tile([128, 5, C2], BF16, tag="w2b")
    nc.sync.dma_start(w2b[:],
                      _ap(w2d, 5 * 128 * C2, [[C2, 128], [128 * C2, 5],
                                              [1, C2]]))

    for n in range(NT):
        ktiles = []
        for kk in range(8):
            t = kst_p.tile([128, 512], BF16, tag="kstream")
            nc.sync.dma_start(
                t[:], _tap(bf_dram, kk * 128 * L + n * 512,
                           [[L, 128], [1, 512]]))
            ktiles.append(t)
        for m in range(4):
            ps = ps_conv.tile([128, 512], F32, tag="conv_ps")
            for k in range(10):
                rhs = (ktiles[k][:] if k < 8
                       else attnT_bf[:, k - 8, n * 512:(n + 1) * 512])
                wt = w2a if k < 5 else w2b
                nc.tensor.matmul(ps[:], wt[:, k % 5, m * 128:(m + 1) * 128],
                                 rhs, start=(k == 0), stop=(k == 9))
            o = st2_p.tile([128, 512], F32, tag="st2", name="o")
            nc.scalar.activation(o[:], ps[:], ACT_MAIN)
            nc.sync.dma_start(
                _ap(outd, m * 128 * L + n * 512, [[L, 128], [1, 512]]), o[:])

    w2_p.release()
    kst_p.release()
    xp_p.release()
    gat_p.release()
    attnT_p.release()
    apt_p.release()
    pre_p.release()
    coef_p.release()
    ctmp_p.release()
    dram_p.release()
    ps_tr.release()
    ps_misc.release()
    ps_conv.release()
    st2_p.release()
    base_p.release()


def host_prep(inputs):
    import ml_dtypes
    x = np.asarray(inputs["x"], np.float32).reshape(B, C1, L).astype(
        ml_dtypes.bfloat16)
    rb = np.asarray(inputs["refer_bbox"], np.float32).reshape(B, L, 2)
    w1t = np.ascontiguousarray(
        np.asarray(inputs["cv1_w"], np.float32)[:, :, 0, 0].T).astype(
            ml_dtypes.bfloat16)
    wc = np.ascontiguousarray(np.stack([
        np.asarray(inputs[k], np.float32).transpose(2, 3, 1, 0).reshape(
            9, C, C)
        for k in ["m0_cv1_w", "m0_cv2_w", "m1_cv1_w", "m1_cv2_w"]])).astype(
            ml_dtypes.bfloat16)
    w2t = np.ascontiguousarray(
        np.asarray(inputs["cv2_w"], np.float32)[:, :, 0, 0].T).astype(
            ml_dtypes.bfloat16)
    out_w = np.ascontiguousarray(
        np.asarray(inputs["out_w"], np.float32)).astype(ml_dtypes.bfloat16)
    shared = {
        "w1t": w1t, "wc": wc, "w2t": w2t, "out_w": out_w,
        "vproj_w": np.ascontiguousarray(
            np.asarray(inputs["vproj_w"], np.float32)).astype(
                ml_dtypes.bfloat16),
        "offaw_w": np.ascontiguousarray(np.concatenate(
            [np.asarray(inputs["off_w"], np.float32),
             np.asarray(inputs["aw_w"], np.float32)], axis=1)).astype(
                ml_dtypes.bfloat16),
        "vproj_b": np.asarray(inputs["vproj_b"], np.float32).reshape(1, D),
        "offaw_b": np.ascontiguousarray(np.concatenate(
            [np.asarray(inputs["off_b"], np.float32),
             np.asarray(inputs["aw_b"], np.float32)]).reshape(1, 96)),
        "out_b": np.asarray(inputs["out_b"], np.float32).reshape(D, 1),
    }
    in_maps = []
    for c in range(B):
        m = dict(shared)
        m["x"] = np.ascontiguousarray(x[c])
        m["refer"] = np.ascontiguousarray(rb[c])
        in_maps.append(m)
    return in_maps


def kernel(**inputs):
    nc = build(B)
    in_maps = host_prep(inputs)
    res = run_bass_kernel_spmd(nc, in_maps, core_ids=list(range(B)))
    out = np.stack([res.results[c]["out"].reshape(C2, H, W) for c in range(B)])
    return out.astype(np.float32)


if __name__ == "__main__":
    build()
    print("build ok")



# revision 18
# speedup vs baseline: 5.5704x; 1.0175x over previous
"""Trainium2 Bass kernel for C2f-with-DeformableAttention block.

Sharding: data-parallel over batch (8 images -> 8 NeuronCores), weights
replicated, no collectives. Each core runs the full block for one image:
  cv1 (1x1) -> split a/b -> 2x Bottleneck(3x3+3x3) -> msdeform attn
  -> concat(a,b,b1,b2,attn) -> cv2 (1x1), SiLU after every conv.

Per-core layouts:
  feature maps: channel-major [C partitions, H*W free]; 3x3-conv inputs are
  zero-padded [C, 66*66] so the 9 taps are contiguous shifted reads feeding
  PSUM-accumulated matmuls.
  deformable sampling: a per-head V4 table in DRAM packs the 4 bilinear
  corner pixels per base index into 512B rows, fetched by SWDGE dma_gather;
  bilinear + softmax attention weights fold into 16 coefficients per sample
  applied with one multiply + one strided reduce on VectorE. Border clipping
  is folded into the coefficients (base index clamped to [0,62]^2, weights
  remapped/zeroed), matching grid_sample(align_corners=False) + masking.
Matmuls run float32r (full-rate fp32 PE mode, fp32 PSUM accumulation); the
concat/cv2 and attention-output paths are bf16. SBUF pressure is managed
with two sequential pool scopes (convs, attention) plus tag-based slot
rotation for the large maps.
"""

import os
import sys

sys.path.insert(0, "/opt/trn_rl_repo")

import numpy as np

import concourse.bass as bass
import concourse.tile as tile
from concourse import bacc, mybir
from concourse.bass import AP
from concourse.bass_utils import run_bass_kernel_spmd
from concourse.masks import make_identity

F32 = mybir.dt.float32
F32R = mybir.dt.float32r
BF16 = mybir.dt.bfloat16
I16 = mybir.dt.int16
ALU = mybir.AluOpType
ACTF = mybir.ActivationFunctionType
AX = mybir.AxisListType

B, C1, C2 = 8, 512, 512
C = 256
D = 256
NH, NP = 8, 4
H = W = 64
L = H * W            # 4096
DH = D // NH         # 32
PW = W + 2           # 66
DOFF = 1             # leading pad element so tap offset -1 stays in-tile
PADLEN = PW * 66 + 16   # per-channel padded map length (+DOFF+tail slack)
OUTREG = 64 * PW     # 4224: contiguous output region = rows 1..64 (all cols)
V4ROWS = 4168        # 66 front pad + 4096 rows + tail
LT = L // 128        # 32
HLT = LT // 2        # 16 (coords run in two l-halves)
NT = L // 512        # 8

SIM_ACT = os.environ.get("BASS_KERNEL_SIM_ACT", "") == "sigmoid"
PH = int(os.environ.get("BASS_KERNEL_PHASES", "9"))
ACT_MAIN = ACTF.Sigmoid if SIM_ACT else ACTF.Silu

_cache = {}


def _ap(t, offset, dims):
    """AP into a DRAM tensor handle at element offset."""
    return AP(t.ap().tensor, offset, dims)


def _tap(tile_, offset, dims):
    """AP into an SBUF/DRAM tile at element offset from tile base."""
    a = tile_[:]
    return AP(a.tensor, a.offset + offset, dims)


def build(n_cores=8):
    key = ("nc", SIM_ACT, PH)
    if key in _cache:
        return _cache[key]
    nc = bacc.Bacc("TRN2", target_bir_lowering=False, debug=False,
                   num_devices=n_cores, num_swdge_queues=4)

    xd = nc.dram_tensor("x", [C1, L], BF16, kind="ExternalInput")
    rbd = nc.dram_tensor("refer", [L, 2], F32, kind="ExternalInput")
    w1d = nc.dram_tensor("w1t", [C1, C1], BF16, kind="ExternalInput")
    wcd = nc.dram_tensor("wc", [4, 9, C, C], BF16, kind="ExternalInput")
    w2d = nc.dram_tensor("w2t", [5 * C, C2], BF16, kind="ExternalInput")
    vpd = nc.dram_tensor("vproj_w", [D, D], BF16, kind="ExternalInput")
    oad = nc.dram_tensor("offaw_w", [D, 96], BF16, kind="ExternalInput")
    owd = nc.dram_tensor("out_w", [D, D], BF16, kind="ExternalInput")
    vbd = nc.dram_tensor("vproj_b", [1, D], F32R, kind="ExternalInput")
    obd = nc.dram_tensor("offaw_b", [1, 96], F32R, kind="ExternalInput")
    wbd = nc.dram_tensor("out_b", [D, 1], F32, kind="ExternalInput")
    outd = nc.dram_tensor("out", [C2, L], F32, kind="ExternalOutput")

    with tile.TileContext(nc) as tc:
        _build_tile(nc, tc, xd, rbd, w1d, wcd, w2d, vpd, oad, owd, vbd, obd,
                    wbd, outd)
    nc.compile()
    _cache[key] = nc
    return nc


def _build_tile(nc, tc, xd, rbd, w1d, wcd, w2d, vpd, oad, owd, vbd, obd, wbd,
                outd):
    def pool(name, bufs, space="SBUF"):
        return tc.alloc_tile_pool(name=name, bufs=bufs, space=space)

    # ---- base pools: live for the whole program ----
    base_p = pool("base", 1)
    st2_p = pool("st2", 3)          # [128,512] staging (spills + outputs)
    ps_conv = pool("ps_conv", 4, space="PSUM")
    ps_misc = pool("ps_misc", 2, space="PSUM")
    ps_tr = pool("ps_tr", 2, space="PSUM")
    dram_p = pool("scratch", 1, space="DRAM")

    ident = base_p.tile([128, 128], F32)
    make_identity(nc, ident[:])
    ident_bf = base_p.tile([128, 128], BF16)
    nc.vector.tensor_copy(ident_bf[:], ident[:])
    ones1 = base_p.tile([1, 128], F32R)
    nc.vector.memset(ones1[:].bitcast(F32), 1.0)
    vb1 = base_p.tile([1, D], F32R)
    nc.sync.dma_start(vb1[:], vbd.ap())
    vbias = base_p.tile([128, D], F32)
    psb = ps_misc.tile([128, 512], F32, tag="psv", name="psb")
    nc.tensor.matmul(psb[:, :D], ones1[:], vb1[:], start=True, stop=True)
    nc.vector.tensor_copy(vbias[:], psb[:, :D])
    ob1 = base_p.tile([1, 96], F32R)
    nc.sync.dma_start(ob1[:], obd.ap())
    obias = base_p.tile([128, 96], F32)
    psb2 = ps_misc.tile([128, 512], F32, tag="psv", name="psb2")
    nc.tensor.matmul(psb2[:, :96], ones1[:], ob1[:], start=True, stop=True)
    nc.vector.tensor_copy(obias[:], psb2[:, :96])
    wbias = base_p.tile([128, 2], F32)
    nc.sync.dma_start(wbias[:], _ap(wbd, 0, [[1, 128], [128, 2]]))
    vproj = base_p.tile([128, 2, D], BF16)
    nc.sync.dma_start(vproj[:], _ap(vpd, 0, [[D, 128], [128 * D, 2], [1, D]]))
    offaw = base_p.tile([128, 2, 96], BF16)
    nc.sync.dma_start(offaw[:],
                      _ap(oad, 0, [[96, 128], [128 * 96, 2], [1, 96]]))
    outw = base_p.tile([128, 2, D], BF16)
    nc.sync.dma_start(outw[:], _ap(owd, 0, [[D, 128], [128 * D, 2], [1, D]]))
    offaw_n = base_p.tile([128, LT, 96], F32)
    rb = base_p.tile([128, LT, 2], F32)
    nc.sync.dma_start(rb[:], _ap(rbd, 0, [[2, 128], [256, LT], [1, 2]]))
    gxb = base_p.tile([128, LT, 2], F32)
    nc.scalar.activation(gxb[:], rb[:], ACTF.Copy, bias=-1.0, scale=64.0)

    bf_dram = dram_p.tile([8, 128, L], BF16)   # a,b,b1,b2 k-tiles for cv2
    v4 = [nc.dram_tensor(f"v4_{h}", [V4ROWS, 128], BF16, kind="Internal")
          for h in range(NH)]
    # idx staging: [8192 rows, 128 cols] i16; read back via xbar transpose
    idx_dram = dram_p.tile([NH * 128 * 1024], I16)

    def spill_chunk(src_ap, slot_k, n):
        """cast a [128,8,64] f32(r) view to bf16 and store to bf_dram."""
        t = st2_p.tile([128, 512], BF16, tag="st2", name="spl")
        dst = _tap(t, 0, [[512, 128], [64, 8], [1, 64]])
        nc.vector.tensor_copy(dst, src_ap)
        nc.sync.dma_start(
            _tap(bf_dram, slot_k * 128 * L + n * 512, [[L, 128], [1, 512]]),
            t[:])

    def spill_map(src_view_fn, slot):
        """spill a 256-ch map (two [128, 64rows, 64] views) to bf_dram."""
        for k in range(2):
            v = src_view_fn(k)
            for n in range(NT):
                sub = AP(v.tensor, v.offset + (n * 8) * v.ap[1][0],
                         [[v.ap[0][0], 128], [v.ap[1][0], 8], [1, 64]])
                spill_chunk(sub, slot * 2 + k, n)

    # ================= scope 1: cv1 + bottlenecks + projections ==========
    s1_p = pool("s1", 1)      # xt
    wc_p = pool("wcp", 2)     # conv weight halves (9KB slots)
    big_p = pool("bigp", 3)   # 17KB bf16 slots: pads, b2, value (rotating)

    xt = s1_p.tile([128, 4, L], BF16, tag="xt")
    for n in range(NT):
        nc.sync.dma_start(
            _tap(xt, n * 512, [[4 * L, 128], [L, 4], [1, 512]]),
            _ap(xd, n * 512, [[L, 128], [128 * L, 4], [1, 512]]))
    w1 = wc_p.tile([128, 4, C1], BF16, tag="wc", name="w1")
    nc.sync.dma_start(w1[:], _ap(w1d, 0, [[C1, 128], [128 * C1, 4], [1, C1]]))

    b_pad = big_p.tile([128, 2, PADLEN], BF16, tag="big", name="b_pad")
    nc.vector.memset(b_pad[:], 0.0)

    for m in range(4):
        for n in range(NT):
            ps = ps_conv.tile([128, 512], F32, tag="conv_ps")
            for k in range(4):
                nc.tensor.matmul(
                    ps[:],
                    w1[:, k, m * 128:(m + 1) * 128],
                    xt[:, k, n * 512:(n + 1) * 512],
                    start=(k == 0), stop=(k == 3))
            if m < 2:
                # 'a' goes straight to DRAM as bf16 (k-tile slot m)
                t = st2_p.tile([128, 512], BF16, tag="st2", name="a_st")
                nc.scalar.activation(t[:], ps[:], ACT_MAIN)
                nc.sync.dma_start(
                    _tap(bf_dram, m * 128 * L + n * 512, [[L, 128], [1, 512]]),
                    t[:])
            else:
                # scatter 512 pixels = 8 rows of 64 into the padded layout
                row0 = n * 8
                dst = _tap(b_pad,
                           (m - 2) * PADLEN + DOFF + (row0 + 1) * PW + 1,
                           [[b_pad[:].ap[0][0], 128], [PW, 8], [1, 64]])
                src = _tap(ps, 0, [[ps[:].ap[0][0], 128], [64, 8], [1, 64]])
                nc.scalar.activation(dst, src, ACT_MAIN)

    # ---- bottleneck convs ----
    wc_tiles = {}

    def load_wc(ci, m):
        t = wc_p.tile([128, 9, 2, 128], BF16, tag="wc", name="wch")
        nc.sync.dma_start(
            t[:], _ap(wcd, ci * 9 * C * C + m * 128,
                      [[C, 128], [C * C, 9], [128 * C, 2], [1, 128]]))
        wc_tiles[(ci, m)] = t

    wcseq = [(ci, m) for ci in range(4) for m in range(2)]
    load_wc(0, 0)

    def conv3x3(src, ci, dst_fn, chunks=None):
        """src: padded [128,2,PADLEN] tile. dst_fn(m, pos, nsz, psum).
        pos/nsz index the 4224-long out region (padded idx DOFF+66+o)."""
        sst = src[:].ap[0][0]
        if chunks is None:
            chunks = [(i * 512, min(512, OUTREG - i * 512)) for i in range(9)]
        for m in range(2):
            wt = wc_tiles[(ci, m)]
            nxt = wcseq.index((ci, m)) + 1
            if nxt < len(wcseq) and wcseq[nxt] not in wc_tiles:
                load_wc(*wcseq[nxt])
            for pos, nsz in chunks:
                ps = ps_conv.tile([128, 512], F32, tag="conv_ps")
                i = 0
                for tap in range(9):
                    ty, tx = tap // 3, tap % 3
                    off = DOFF + pos + ty * PW + tx - 1
                    for k in range(2):
                        nc.tensor.matmul(
                            ps[:, :nsz],
                            wt[:, tap, k, :],
                            _tap(src, k * PADLEN + off,
                                 [[sst, 128], [1, nsz]]),
                            start=(i == 0), stop=(i == 17))
                        i += 1
                dst_fn(m, pos, nsz, ps)

    def pad_writer(dst):
        def f(m, pos, nsz, ps):
            nc.scalar.activation(
                _tap(dst, m * PADLEN + DOFF + PW + pos,
                     [[dst[:].ap[0][0], 128], [1, nsz]]),
                ps[:, :nsz], ACT_MAIN)
        return f

    def zero_padcols(t):
        nc.vector.memset(
            _tap(t, DOFF + PW, [[t[:].ap[0][0], 128], [PADLEN, 2], [PW, 64],
                                [65, 2]]), 0.0)

    def padded_view(t, k):
        return _tap(t, k * PADLEN + DOFF + PW + 1,
                    [[t[:].ap[0][0], 128], [PW, 64], [1, 64]])

    mid = big_p.tile([128, 2, PADLEN], BF16, tag="big", name="mid")
    nc.vector.memset(mid[:], 0.0)
    conv3x3(b_pad, 0, pad_writer(mid))
    zero_padcols(mid)
    spill_map(lambda k: padded_view(b_pad, k), 1)

    b1_pad = big_p.tile([128, 2, PADLEN], BF16, tag="big", name="b1_pad")
    nc.vector.memset(b1_pad[:], 0.0)
    conv3x3(mid, 1, pad_writer(b1_pad))
    zero_padcols(b1_pad)

    mid2 = big_p.tile([128, 2, PADLEN], BF16, tag="big", name="mid2")
    nc.vector.memset(mid2[:], 0.0)
    conv3x3(b1_pad, 2, pad_writer(mid2))
    zero_padcols(mid2)
    spill_map(lambda k: padded_view(b1_pad, k), 2)

    b2 = big_p.tile([128, 2, PADLEN], BF16, tag="big", name="b2")

    def b2_writer(m, pos, nsz, ps):
        row0, nrow = pos // PW, nsz // PW
        dst = _tap(b2, m * PADLEN + row0 * 64,
                   [[b2[:].ap[0][0], 128], [64, nrow], [1, 64]])
        src = _tap(ps, 1, [[ps[:].ap[0][0], 128], [PW, nrow], [1, 64]])
        nc.scalar.activation(dst, src, ACT_MAIN)

    # conv4 runs chunk-major fused with the per-l-tile projections so
    # value/offsets materialize incrementally: coords (DVE), the v4 table
    # build (DMA) and the idx pipeline all overlap the conv tail instead of
    # serializing after it.
    value = big_p.tile([128, 2, PADLEN], BF16, tag="big", name="value")
    vst = value[:].ap[0][0]
    load_wc(3, 1)
    sst4 = mid2[:].ap[0][0]
    b2st = b2[:].ap[0][0]
    for rc in range(16):
        pos, nsz = rc * 4 * PW, 4 * PW
        for m in range(2):
            wt = wc_tiles[(3, m)]
            ps = ps_conv.tile([128, 512], F32, tag="conv_ps")
            i = 0
            for tap in range(9):
                ty, tx = tap // 3, tap % 3
                off = DOFF + pos + ty * PW + tx - 1
                for k in range(2):
                    nc.tensor.matmul(
                        ps[:, :nsz], wt[:, tap, k, :],
                        _tap(mid2, k * PADLEN + off, [[sst4, 128], [1, nsz]]),
                        start=(i == 0), stop=(i == 17))
                    i += 1
            b2_writer(m, pos, nsz, ps)
        for lt in (2 * rc, 2 * rc + 1):
            psv = ps_misc.tile([128, 512], F32, tag="psv")
            for k in range(2):
                nc.tensor.matmul(psv[:, :D],
                                 _tap(b2, k * PADLEN + lt * 128,
                                      [[b2st, 128], [1, 128]]),
                                 vproj[:, k, :],
                                 start=(k == 0), stop=(k == 1))
            nc.vector.tensor_tensor(_tap(value, lt * D, [[vst, 128], [1, D]]),
                                    psv[:, :D], vbias[:], ALU.add)
            pso = ps_misc.tile([128, 512], F32, tag="psv")
            for k in range(2):
                nc.tensor.matmul(pso[:, :96],
                                 _tap(b2, k * PADLEN + lt * 128,
                                      [[b2st, 128], [1, 128]]),
                                 offaw[:, k, :],
                                 start=(k == 0), stop=(k == 1))
            nc.vector.tensor_tensor(offaw_n[:, lt, :], pso[:, :96], obias[:],
                                    ALU.add)

    if PH < 2:
        big_p.release(); wc_p.release(); s1_p.release()
        dram_p.release(); ps_tr.release(); ps_misc.release()
        ps_conv.release(); st2_p.release(); base_p.release()
        return
    spill_map(lambda k: _tap(b2, k * PADLEN,
                             [[b2[:].ap[0][0], 128], [64, 64], [1, 64]]), 3)

    # V4: per head h, row (66+i) = [V[i], V[i+1], V[i+64], V[i+65]].
    # Issued in l-halves so the first half streams while conv4 still runs.
    for hh in range(NH):
        for si, dlt in enumerate([0, 1, 64, 65]):
            for lh in range(2):
                src = _tap(value, hh * DH + lh * HLT * D,
                           [[vst, 128], [D, HLT], [1, DH]])
                dst = _ap(v4[hh],
                          (66 - dlt + lh * HLT * 128) * 128 + si * DH,
                          [[128, 128], [128 * 128, HLT], [1, DH]])
                nc.sync.dma_start(dst, src)

    big_p.release()
    wc_p.release()
    s1_p.release()

    if PH < 3:
        dram_p.release(); ps_tr.release(); ps_misc.release()
        ps_conv.release(); st2_p.release(); base_p.release()
        return

    # ================= scope 2: coords + gather + attn + cv2 =============
    ctmp_p = pool("ctmp", 1)
    coef_p = pool("coefp", 1)
    pre_p = pool("prep", 1)
    apt_p = pool("aptp", 2)
    attnT_p = pool("attnTp", 1)
    gat_p = pool("gatp", 4)
    xp_p = pool("xp", 2)
    kst_p = pool("kst", 6)
    w2_p = pool("w2p", 1)

    coef = coef_p.tile([128, NH, LT, NP, 4], BF16, tag="coef")
    cst = coef[:].ap[0][0]
    idxF = coef_p.tile([128, NH, LT, NP], F32, tag="idxF")
    ist = idxF[:].ap[0][0]
    # wrapped-idx staging: free dim = (lH 8, replica 8, j 16) per head
    t16 = coef_p.tile([128, NH, 8, 8, 16], I16, tag="t16")
    idx_wr = coef_p.tile([128, NH * 1024], I16, tag="idx_wr")

    # ---- sampling coordinates, in two l-halves ----
    SH = [128, HLT, 32]
    ost = offaw_n[:].ap[0][0]

    def lhc(t):
        """[128, HLT, 32combo] tile viewed as [part, lt, h, p]."""
        return _tap(t, 0, [[t[:].ap[0][0], 128], [32, HLT], [NP, NH],
                           [1, NP]])

    cp = ctmp_p
    for lh in range(2):
        lt0 = lh * HLT

        def off_view(xy):
            return _tap(offaw_n, lt0 * 96 + xy,
                        [[ost, 128], [96, HLT], [2, 32]])

        def axis_weights(xy, lim):
            g = cp.tile(SH, F32, tag="g")
            gb = _tap(gxb, lt0 * 2 + xy,
                      [[gxb[:].ap[0][0], 128], [2, HLT], [0, 32]])
            nc.vector.tensor_tensor(g[:], off_view(xy), gb, ALU.add)
            # g holds g_true-0.5 (gxb bias -1.0 = grid's -0.5 plus -0.5
            # for round->floor). x0 = round(g) = floor(g_true) via the fp32
            # magic constant (2^23*1.5, representable; at g_true exactly
            # integer the half-even tie gives floor or floor-1, both of
            # which produce identical interpolation).
            x0 = cp.tile(SH, F32, tag="x0")
            nc.vector.tensor_scalar(x0[:], g[:], 12582912.0, 12582912.0,
                                    ALU.add, ALU.subtract)
            fr = cp.tile(SH, F32, tag="t1", name="fr")
            nc.vector.tensor_tensor(fr[:], g[:], x0[:], ALU.subtract)
            wfrac = cp.tile(SH, F32, tag="wf")
            nc.vector.tensor_scalar(wfrac[:], fr[:], 0.5, None, ALU.add)
            wcmp = cp.tile(SH, F32, tag="wcm")
            nc.vector.tensor_scalar(wcmp[:], fr[:], -1.0, 0.5, ALU.mult,
                                    ALU.add)
            bx = cp.tile(SH, F32, tag=f"bx{xy}")
            nc.vector.tensor_scalar(bx[:], x0[:], 0.0, float(lim), ALU.max,
                                    ALU.min)
            d = cp.tile(SH, F32, tag="d")
            nc.vector.tensor_tensor(d[:], bx[:], x0[:], ALU.subtract)
            e0 = cp.tile(SH, F32, tag="e0")
            nc.vector.tensor_scalar(e0[:], d[:], 0.0, None, ALU.is_equal)
            em = cp.tile(SH, F32, tag="em")
            nc.vector.tensor_scalar(em[:], d[:], 1.0, None, ALU.is_equal)
            ep = cp.tile(SH, F32, tag="ep")
            nc.vector.tensor_scalar(ep[:], d[:], -1.0, None, ALU.is_equal)
            t1 = cp.tile(SH, F32, tag="t1")
            s0 = cp.tile(SH, F32, tag=f"s0{xy}")
            nc.vector.tensor_tensor(t1[:], e0[:], wcmp[:], ALU.mult)
            nc.vector.tensor_tensor(s0[:], em[:], wfrac[:], ALU.mult)
            nc.vector.tensor_tensor(s0[:], s0[:], t1[:], ALU.add)
            s1 = cp.tile(SH, F32, tag=f"s1{xy}")
            nc.vector.tensor_tensor(t1[:], e0[:], wfrac[:], ALU.mult)
            nc.vector.tensor_tensor(s1[:], ep[:], wcmp[:], ALU.mult)
            nc.vector.tensor_tensor(s1[:], s1[:], t1[:], ALU.add)
            return s0, s1, bx

        sx0, sx1, bxx = axis_weights(0, W - 2)
        sy0, sy1, bxy = axis_weights(1, H - 2)

        idxf = cp.tile(SH, F32, tag="g", name="idxf")
        nc.vector.tensor_scalar(idxf[:], bxy[:], float(W), 66.0, ALU.mult,
                                ALU.add)
        nc.vector.tensor_tensor(idxf[:], idxf[:], bxx[:], ALU.add)
        idx_dst = _tap(idxF, lt0 * NP,
                       [[ist, 128], [NP, HLT], [LT * NP, NH], [1, NP]])
        nc.vector.tensor_copy(idx_dst, lhc(idxf))

        # softmax over p
        aw4 = _tap(offaw_n, lt0 * 96 + 64,
                   [[ost, 128], [96, HLT], [4, NH], [1, NP]])
        mx = cp.tile([128, HLT, NH], F32, tag="em", name="mx")
        nc.vector.tensor_reduce(mx[:], aw4, AX.X, ALU.max)
        mxb = _tap(mx, 0, [[mx[:].ap[0][0], 128], [NH, HLT], [1, NH],
                           [0, NP]])
        z = cp.tile(SH, F32, tag="x0", name="z")
        zv = _tap(z, 0, [[z[:].ap[0][0], 128], [32, HLT], [4, NH], [1, NP]])
        nc.vector.tensor_tensor(zv, aw4, mxb, ALU.subtract)
        ez = cp.tile(SH, F32, tag="d", name="ez")
        nc.scalar.activation(ez[:], z[:], ACTF.Exp)
        ezv = _tap(ez, 0, [[ez[:].ap[0][0], 128], [32, HLT], [4, NH],
                           [1, NP]])
        ssum = cp.tile([128, HLT, NH], F32, tag="ep", name="ssum")
        nc.vector.tensor_reduce(ssum[:], ezv, AX.X, ALU.add)
        rs = cp.tile([128, HLT, NH], F32, tag="t1", name="rs")
        nc.vector.reciprocal(rs[:], ssum[:])
        rsb = _tap(rs, 0, [[rs[:].ap[0][0], 128], [NH, HLT], [1, NH],
                           [0, NP]])
        Aw = cp.tile(SH, F32, tag="e0", name="Aw")
        Av = _tap(Aw, 0, [[Aw[:].ap[0][0], 128], [32, HLT], [4, NH], [1, NP]])
        nc.vector.tensor_tensor(Av, ezv, rsb, ALU.mult)

        tprod = cp.tile(SH, F32, tag="wf", name="tprod")
        for slot, (sa, sb) in enumerate([(sx0, sy0), (sx1, sy0), (sx0, sy1),
                                         (sx1, sy1)]):
            nc.vector.tensor_tensor(tprod[:], sa[:], sb[:], ALU.mult)
            cdst = AP(coef[:].tensor, coef[:].offset + lt0 * NP * 4 + slot,
                      [[cst, 128], [NP * 4, HLT], [LT * NP * 4, NH], [4, NP]])
            nc.vector.tensor_tensor(cdst, lhc(tprod), lhc(Aw), ALU.mult)

    # ---- idx shuffle: PE transpose -> replicated i16 -> DRAM -> xbar read
    # For head h the gather consumes stream i = lt*512 + pp*128 + l_lo in a
    # 16-wrap [j=i%16 partition, c=i//16 col]. Transposing idxF puts (lt,pp)
    # on partitions; the l_lo free dim then splits as (lH, j) and a stride-0
    # read replicates the 16-value groups for the 8 Q7 core pairs. One
    # contiguous 2KB-run DMA stages [8192, 128] i16 in DRAM; one hardware
    # xbar transpose lands the wrapped layout in SBUF.
    tst = t16[:].ap[0][0]
    for h in range(NH):
        pst = ps_tr.tile([128, 128], F32, tag="pst", name="pidx")
        nc.tensor.transpose(
            pst[:], _tap(idxF, h * LT * NP, [[ist, 128], [NP, LT], [1, NP]]),
            ident[:])
        nc.vector.tensor_copy(
            _tap(t16, h * 1024, [[tst, 128], [128, 8], [16, 8], [1, 16]]),
            _tap(pst, 0, [[pst[:].ap[0][0], 128], [16, 8], [0, 8], [1, 16]]))
    nc.sync.dma_start(
        _tap(idx_dram, 0, [[1024, 128], [128 * 1024, NH], [1, 1024]]),
        t16[:])
    nc.sync.dma_start_transpose(
        idx_wr[:], _tap(idx_dram, 0, [[128, NH * 1024], [1, 128]]))

    if PH < 4:
        w2_p.release(); kst_p.release(); xp_p.release()
        gat_p.release()
        attnT_p.release(); apt_p.release(); pre_p.release()
        coef_p.release(); ctmp_p.release(); dram_p.release()
        ps_tr.release(); ps_misc.release(); ps_conv.release()
        st2_p.release(); base_p.release()
        return

    # ---- gather + weighted reduce + transpose ----
    attn_preT = []
    for hg in range(2):
        pre = pre_p.tile([128, LT, 4, DH], F32, tag="pre")
        prest = pre[:].ap[0][0]
        for hi in range(4):
            h = hg * 4 + hi
            for q in range(4):      # quarter-head chunks: 8 l-tiles each
                g = gat_p.tile([128, 32, 128], BF16, tag="gat")
                idxs = idx_wr[:, h * 1024 + q * 256:h * 1024 + (q + 1) * 256]
                nc.gpsimd.dma_gather(
                    g[:],
                    _ap(v4[h], 0, [[128, 4097], [1, 128]]),
                    idxs, 4096, 4096, 128, single_packet=False,
                    queue_num=q)
                gst = g[:].ap[0][0]
                gv = _tap(g, 0, [[gst, 128], [512, 8], [128, NP], [32, 4],
                                 [1, DH]])
                cch = AP(coef[:].tensor,
                         coef[:].offset + h * LT * NP * 4 + q * 8 * NP * 4,
                         [[cst, 128], [16, 8], [4, NP], [1, 4], [0, DH]])
                nc.vector.tensor_tensor(gv, gv, cch, ALU.mult)
                # pairwise-add tree over the 16 (point, corner) slots:
                # bf16 tensor_tensor runs 2x/cycle; tensor_reduce is 1x-only.
                x1 = xp_p.tile([128, 8, 8, DH], BF16, tag="x1")
                s1 = x1[:].ap[0][0]
                nc.vector.tensor_tensor(
                    x1[:],
                    _tap(g, 0, [[gst, 128], [512, 8], [64, 8], [1, DH]]),
                    _tap(g, DH, [[gst, 128], [512, 8], [64, 8], [1, DH]]),
                    ALU.add)
                x2 = xp_p.tile([128, 8, 4, DH], BF16, tag="x2")
                s2 = x2[:].ap[0][0]
                nc.vector.tensor_tensor(
                    x2[:],
                    _tap(x1, 0, [[s1, 128], [256, 8], [64, 4], [1, DH]]),
                    _tap(x1, DH, [[s1, 128], [256, 8], [64, 4], [1, DH]]),
                    ALU.add)
                x3 = xp_p.tile([128, 8, 2, DH], BF16, tag="x3")
                s3 = x3[:].ap[0][0]
                nc.vector.tensor_tensor(
                    x3[:],
                    _tap(x2, 0, [[s2, 128], [128, 8], [64, 2], [1, DH]]),
                    _tap(x2, DH, [[s2, 128], [128, 8], [64, 2], [1, DH]]),
                    ALU.add)
                nc.vector.tensor_tensor(
                    _tap(pre, (q * 8) * 4 * DH + hi * DH,
                         [[prest, 128], [4 * DH, 8], [1, DH]]),
                    _tap(x3, 0, [[s3, 128], [64, 8], [1, DH]]),
                    _tap(x3, DH, [[s3, 128], [64, 8], [1, DH]]),
                    ALU.add)
        # transpose [128 l_lo, 128 (4 heads x 32 dh)] per l-tile -> bf16
        apt = apt_p.tile([128, L], BF16, tag="apT", name="apt")
        attn_preT.append(apt)
        for lt in range(LT):
            pst = ps_tr.tile([128, 128], F32, tag="pst")
            nc.tensor.transpose(pst[:], pre[:, lt, :, :], ident[:])
            nc.vector.tensor_copy(apt[:, lt * 128:(lt + 1) * 128], pst[:])

    if PH < 5:
        w2_p.release(); kst_p.release(); xp_p.release()
        gat_p.release()
        attnT_p.release(); apt_p.release(); pre_p.release()
        coef_p.release(); ctmp_p.release(); dram_p.release()
        ps_tr.release(); ps_misc.release(); ps_conv.release()
        st2_p.release(); base_p.release()
        return

    # ---- cv2 partials over the 8 pre-attention k-tiles ----
    # These only depend on the spilled maps, so the Tile scheduler runs them
    # during the gather phase where the PE is otherwise idle. Partials are
    # staged to DRAM in bf16; the tail re-loads them into PSUM via an
    # identity matmul and adds just the attention contribution.
    w2a = w2_p.tile([128, 5, C2], BF16, tag="w2a")
    nc.sync.dma_start(w2a[:], _ap(w2d, 0, [[C2, 128], [128 * C2, 5], [1, C2]]))
    w2b = w2_p.tile([128, 5, C2], BF16, tag="w2b")
    nc.sync.dma_start(w2b[:],
                      _ap(w2d, 5 * 128 * C2, [[C2, 128], [128 * C2, 5],
                                              [1, C2]]))
    part_dram = dram_p.tile([4, 128, L], BF16)
    for n in range(NT):
        ktiles = []
        for kk in range(8):
            t = kst_p.tile([128, 512], BF16, tag="kstream")
            nc.sync.dma_start(
                t[:], _tap(bf_dram, kk * 128 * L + n * 512,
                           [[L, 128], [1, 512]]))
            ktiles.append(t)
        for m in range(4):
            ps = ps_conv.tile([128, 512], F32, tag="conv_ps")
            for k in range(8):
                wt = w2a if k < 5 else w2b
                nc.tensor.matmul(ps[:], wt[:, k % 5, m * 128:(m + 1) * 128],
                                 ktiles[k][:], start=(k == 0), stop=(k == 7))
            t2 = st2_p.tile([128, 512], BF16, tag="st2", name="part")
            nc.vector.tensor_copy(t2[:], ps[:])
            nc.sync.dma_start(
                _tap(part_dram, m * 128 * L + n * 512, [[L, 128], [1, 512]]),
                t2[:])

    # ---- attn out-projection (bf16, +out_b) ----
    attnT_bf = attnT_p.tile([128, 2, L], BF16, tag="attnT")
    for mg in range(2):
        for n in range(NT):
            ps = ps_misc.tile([128, 512], F32, tag="psv")
            for k in range(2):
                nc.tensor.matmul(
                    ps[:],
                    outw[:, k, mg * 128:(mg + 1) * 128],
                    attn_preT[k][:, n * 512:(n + 1) * 512],
                    start=(k == 0), stop=(k == 1))
            nc.scalar.activation(attnT_bf[:, mg, n * 512:(n + 1) * 512],
                                 ps[:], ACTF.Identity,
                                 bias=wbias[:, mg:mg + 1])

    # ---- cv2 tail: partial (PSUM preload) + attention k-tiles ----
    for n in range(NT):
        for m in range(4):
            pt = kst_p.tile([128, 512], BF16, tag="kpart")
            nc.sync.dma_start(
                pt[:], _tap(part_dram, m * 128 * L + n * 512,
                            [[L, 128], [1, 512]]))
            ps = ps_conv.tile([128, 512], F32, tag="conv_ps")
            nc.tensor.matmul(ps[:], ident_bf[:], pt[:],
                             start=True, stop=False)
            for k in (8, 9):
                nc.tensor.matmul(ps[:], w2b[:, k - 5, m * 128:(m + 1) * 128],
                                 attnT_bf[:, k - 8, n * 512:(n + 1) * 512],
                                 start=False, stop=(k == 9))
            o = st2_p.tile([128, 512], F32, tag="st2", name="o")
            nc.scalar.activation(o[:], ps[:], ACT_MAIN)
            nc.sync.dma_start(
                _ap(outd, m * 128 * L + n * 512, [[L, 128], [1, 512]]), o[:])

    w2_p.release()
    kst_p.release()
    xp_p.release()
    gat_p.release()
    attnT_p.release()
    apt_p.release()
    pre_p.release()
    coef_p.release()
    ctmp_p.release()
    dram_p.release()
    ps_tr.release()
    ps_misc.release()
    ps_conv.release()
    st2_p.release()
    base_p.release()


def host_prep(inputs):
    import ml_dtypes
    x = np.asarray(inputs["x"], np.float32).reshape(B, C1, L).astype(
        ml_dtypes.bfloat16)
    rb = np.asarray(inputs["refer_bbox"], np.float32).reshape(B, L, 2)
    w1t = np.ascontiguousarray(
        np.asarray(inputs["cv1_w"], np.float32)[:, :, 0, 0].T).astype(
            ml_dtypes.bfloat16)
    wc = np.ascontiguousarray(np.stack([
        np.asarray(inputs[k], np.float32).transpose(2, 3, 1, 0).reshape(
            9, C, C)
        for k in ["m0_cv1_w", "m0_cv2_w", "m1_cv1_w", "m1_cv2_w"]])).astype(
            ml_dtypes.bfloat16)
    w2t = np.ascontiguousarray(
        np.asarray(inputs["cv2_w"], np.float32)[:, :, 0, 0].T).astype(
            ml_dtypes.bfloat16)
    out_w = np.ascontiguousarray(
        np.asarray(inputs["out_w"], np.float32)).astype(ml_dtypes.bfloat16)
    shared = {
        "w1t": w1t, "wc": wc, "w2t": w2t, "out_w": out_w,
        "vproj_w": np.ascontiguousarray(
            np.asarray(inputs["vproj_w"], np.float32)).astype(
                ml_dtypes.bfloat16),
        "offaw_w": np.ascontiguousarray(np.concatenate(
            [np.asarray(inputs["off_w"], np.float32),
             np.asarray(inputs["aw_w"], np.float32)], axis=1)).astype(
                ml_dtypes.bfloat16),
        "vproj_b": np.asarray(inputs["vproj_b"], np.float32).reshape(1, D),
        "offaw_b": np.ascontiguousarray(np.concatenate(
            [np.asarray(inputs["off_b"], np.float32),
             np.asarray(inputs["aw_b"], np.float32)]).reshape(1, 96)),
        "out_b": np.asarray(inputs["out_b"], np.float32).reshape(D, 1),
    }
    in_maps = []
    for c in range(B):
        m = dict(shared)
        m["x"] = np.ascontiguousarray(x[c])
        m["refer"] = np.ascontiguousarray(rb[c])
        in_maps.append(m)
    return in_maps


def kernel(**inputs):
    nc = build(B)
    in_maps = host_prep(inputs)
    res = run_bass_kernel_spmd(nc, in_maps, core_ids=list(range(B)))
    out = np.stack([res.results[c]["out"].reshape(C2, H, W) for c in range(B)])
    return out.astype(np.float32)


if __name__ == "__main__":
    build()
    print("build ok")

